# revision 1
# baseline (speedup 1.0000x reference)
"""Trainium2 Bass kernel for nn_ModalGenerator (MoE-routed cross-modal generator).

Strategy:
  - seq_len==1 => softmax over one key is identically 1, so attention output is
    just v = tgt @ wv.T + bv. Fold wv/ao_w into one 512x512 matrix per layer
    (host-side), and fold (1-rw) into the output projection.
  - MoE routing: only columns with missing_type==1 need generator 0 (img->text)
    and only missing_type==2 need generator 1 (text->img). Host gathers those
    columns, the device runs the generators on the compacted columns only
    (~1/4 of the batch each), host scatters results back. missing_type==3 rows
    use the (host-computed, tiny) prior MLP; other rows pass through.
  - Data-parallel over 8 NeuronCores: columns sharded, weights replicated.
  - Activations live transposed [H(partitions), cols(free)]. All matmuls in
    float32r (e8m11) at full PE rate. LayerNorm stats via ones-matmul (column
    sums, already broadcast across partitions), rsqrt via bit-hack + Newton on
    VectorE, exact Gelu on ScalarE (single ACT table set for the whole kernel).
"""

import math

import numpy as np

import concourse.bacc as bacc
import concourse.mybir as mybir
import concourse.tile as tile
from concourse.bass_utils import run_bass_kernel_spmd

f32 = mybir.dt.float32
f32r = mybir.dt.float32r
i32 = mybir.dt.int32
AF = mybir.ActivationFunctionType
ALU = mybir.AluOpType

H = 512
L = 3
N_CORES = 8
KC = H // 128            # 4 K-chunks of the hidden dim
FH = 4 * H               # 2048 FFN hidden
FKC = FH // 128          # 16
LN_EPS = 1e-5
MAGIC = 0x5F3759DF

# param pack column layout (per generator): [128, 128] f32
_P_IPB = 0
_P_LAYER = 4             # + 40*l: ba 0..3 | f1b 4..19 | f2b 20..23
#                                 | ln1g 24..27 | ln1b 28..31 | ln2g 32..35 | ln2b 36..39
_P_OPB = 124


def _round_f32r(a):
    """Round-to-nearest-even fp32 -> fp32r (e8m11: low 12 mantissa bits zero)."""
    b = np.ascontiguousarray(a, dtype=np.float32).view(np.uint32).copy()
    b += np.uint32(0x7FF) + ((b >> np.uint32(12)) & np.uint32(1))
    b &= np.uint32(0xFFFFF000)
    return b.view(np.float32)


def _pack_pcol(vec):
    """[n*128] vector -> [128, n] chunk-column layout."""
    return np.ascontiguousarray(np.asarray(vec, np.float32).reshape(-1, 128).T)


def _sb_pack(wT):
    """[K, M] (K mult of 128) -> [128, (K/128)*M] SBUF chunk-major layout."""
    K, M = wT.shape
    return np.ascontiguousarray(
        wT.reshape(K // 128, 128, M).transpose(1, 0, 2).reshape(128, -1))


def _ntiles(C):
    if C <= 512:
        return [(0, C)]
    h = ((C // 2) + 31) // 32 * 32
    return [(0, h), (h, C)]


def _build_program(C0, C1, skip_b, repeat=1):
    nc = bacc.Bacc("TRN2", target_bir_lowering=False, debug=False,
                   num_devices=N_CORES)

    dram = {}
    for g, C in ((0, C0), (1, C1)):
        dram[f"src{g}"] = nc.dram_tensor(f"src{g}", [128, KC * C], f32r, kind="ExternalInput")
        dram[f"tgt{g}"] = nc.dram_tensor(f"tgt{g}", [128, KC * C], f32r, kind="ExternalInput")
        dram[f"io{g}"] = nc.dram_tensor(f"io{g}", [128, 2 * KC * H], f32r, kind="ExternalInput")
        dram[f"wa{g}"] = nc.dram_tensor(f"wa{g}", [L, 128, KC * H], f32r, kind="ExternalInput")
        dram[f"f1{g}"] = nc.dram_tensor(f"f1{g}", [L, 128, KC * FH], f32r, kind="ExternalInput")
        dram[f"f2{g}"] = nc.dram_tensor(f"f2{g}", [L, 128, FKC * H], f32r, kind="ExternalInput")
        dram[f"par{g}"] = nc.dram_tensor(f"par{g}", [128, 128], f32, kind="ExternalInput")
        dram[f"out{g}"] = nc.dram_tensor(f"out{g}", [128, KC * C], f32, kind="ExternalOutput")
    dram["ones"] = nc.dram_tensor("ones", [128, 128], f32r, kind="ExternalInput")

    h_bufs = 2 if max(C0, C1) <= 512 else 1

    with tile.TileContext(nc) as tc:
        with (
            tc.tile_pool(name="sb", bufs=2) as sb,
            tc.tile_pool(name="ps", bufs=4, space="PSUM") as psp,
        ):
            ones = sb.tile([128, 128], f32r, tag="ones", bufs=1)
            nc.sync.dma_start(ones[:], dram["ones"].ap())

            def ln_stats(y, cs):
                """column sums of y and y^2 (broadcast over partitions) -> rstd, m."""
                c0, c1 = cs
                Ct = c1 - c0
                s_ps = psp.tile([128, Ct], f32, tag="s", bufs=2)
                q_ps = psp.tile([128, Ct], f32, tag="q", bufs=2)
                for k in range(KC):
                    nc.tensor.matmul(s_ps[:], ones[:], y[:, k, c0:c1],
                                     start=(k == 0), stop=(k == KC - 1))
                for k in range(KC):
                    ysq = sb.tile([128, Ct], f32r, tag="ysq")
                    nc.scalar.activation(ysq[:], y[:, k, c0:c1], AF.Square)
                    nc.tensor.matmul(q_ps[:], ones[:], ysq[:],
                                     start=(k == 0), stop=(k == KC - 1))
                m_bc = sb.tile([128, Ct], f32, tag="m")
                nc.vector.tensor_scalar(m_bc[:], s_ps[:], 1.0 / H, None, ALU.mult)
                msq = sb.tile([128, Ct], f32, tag="msq", bufs=1)
                nc.vector.tensor_mul(msq[:], m_bc[:], m_bc[:])
                z = sb.tile([128, Ct], f32, tag="z")
                nc.vector.scalar_tensor_tensor(z[:], q_ps[:], 1.0 / H, msq[:],
                                               ALU.mult, ALU.subtract)
                nc.vector.tensor_scalar(z[:], z[:], LN_EPS, None, ALU.add)
                ti = sb.tile([128, Ct], i32, tag="ti", bufs=1)
                nc.vector.tensor_scalar(ti[:], z[:].bitcast(i32), 1, None,
                                        ALU.arith_shift_right)
                rstd = sb.tile([128, Ct], f32, tag="rstd", bufs=2)
                nc.vector.tensor_scalar(rstd[:].bitcast(i32), ti[:], -1, MAGIC,
                                        ALU.mult, ALU.add)
                for _ in range(2):
                    u = sb.tile([128, Ct], f32, tag="u", bufs=1)
                    nc.vector.tensor_mul(u[:], rstd[:], rstd[:])
                    w = sb.tile([128, Ct], f32, tag="w", bufs=1)
                    nc.vector.scalar_tensor_tensor(w[:], u[:], -0.5, z[:],
                                                   ALU.mult, ALU.mult)
                    rstd2 = sb.tile([128, Ct], f32, tag="rstd", bufs=2)
                    nc.vector.scalar_tensor_tensor(rstd2[:], w[:], 1.5, rstd[:],
                                                   ALU.add, ALU.mult)
                    rstd = rstd2
                return m_bc, rstd

            def ln_apply(y, xn, cs, par, gcol, bcol, m_bc, rstd, skip_beta):
                c0, c1 = cs
                for m in range(KC):
                    u1 = sb.tile([128, c1 - c0], f32, tag="u1")
                    nc.vector.tensor_sub(u1[:], y[:, m, c0:c1], m_bc[:])
                    if skip_beta:
                        nc.vector.scalar_tensor_tensor(
                            xn[:, m, c0:c1], u1[:], par[:, gcol + m:gcol + m + 1],
                            rstd[:], ALU.mult, ALU.mult)
                    else:
                        u2 = sb.tile([128, c1 - c0], f32, tag="u2", bufs=1)
                        nc.vector.scalar_tensor_tensor(
                            u2[:], u1[:], par[:, gcol + m:gcol + m + 1],
                            rstd[:], ALU.mult, ALU.mult)
                        nc.vector.tensor_scalar(
                            xn[:, m, c0:c1], u2[:], par[:, bcol + m:bcol + m + 1],
                            None, ALU.add)

            for _rep in range(repeat):
             for g, C in ((0, C0), (1, C1)):
                tiles = _ntiles(C)
                src = sb.tile([128, KC, C], f32r, tag="x", bufs=2)
                tgt = sb.tile([128, KC, C], f32r, tag="tgt", bufs=1)
                nc.sync.dma_start(src[:], dram[f"src{g}"].ap())
                nc.sync.dma_start(tgt[:], dram[f"tgt{g}"].ap())
                par = sb.tile([128, 128], f32, tag="par", bufs=1)
                nc.sync.dma_start(par[:], dram[f"par{g}"].ap())
                iow = sb.tile([128, KC * H], f32r, tag="io", bufs=1)
                nc.sync.dma_start(iow[:], dram[f"io{g}"].ap()[:, 0:KC * H])

                # input proj: x = ipwT.T @ src + ipb
                x = sb.tile([128, KC, C], f32r, tag="x", bufs=2)
                for cs in tiles:
                    for m in range(KC):
                        ps = psp.tile([128, cs[1] - cs[0]], f32, tag="mm")
                        for k in range(KC):
                            nc.tensor.matmul(
                                ps[:], iow[:, k * H + 128 * m:k * H + 128 * (m + 1)],
                                src[:, k, cs[0]:cs[1]],
                                start=(k == 0), stop=(k == KC - 1))
                        nc.vector.tensor_scalar(
                            x[:, m, cs[0]:cs[1]], ps[:],
                            par[:, _P_IPB + m:_P_IPB + m + 1], None, ALU.add)

                for l in range(L):
                    pb = _P_LAYER + 40 * l
                    wa = sb.tile([128, KC * H], f32r, tag="wa", bufs=2)
                    nc.sync.dma_start(wa[:], dram[f"wa{g}"].ap()[l])
                    f1w = sb.tile([128, KC * FH], f32r, tag="f1", bufs=1)
                    nc.sync.dma_start(f1w[:], dram[f"f1{g}"].ap()[l])
                    f2w = sb.tile([128, FKC, H], f32r, tag="f2", bufs=1)
                    nc.sync.dma_start(f2w[:], dram[f"f2{g}"].ap()[l])

                    # ---- attention(=v proj) + residual + LN1 ----
                    xn = sb.tile([128, KC, C], f32r, tag="x", bufs=2)
                    for cs in tiles:
                        y = sb.tile([128, KC, C], f32r, tag="y")
                        for m in range(KC):
                            ps = psp.tile([128, cs[1] - cs[0]], f32, tag="mm")
                            for k in range(KC):
                                nc.tensor.matmul(
                                    ps[:], wa[:, k * H + 128 * m:k * H + 128 * (m + 1)],
                                    tgt[:, k, cs[0]:cs[1]],
                                    start=(k == 0), stop=(k == KC - 1))
                            nc.vector.scalar_tensor_tensor(
                                y[:, m, cs[0]:cs[1]], ps[:],
                                par[:, pb + m:pb + m + 1], x[:, m, cs[0]:cs[1]],
                                ALU.add, ALU.add)
                        m_bc, rstd = ln_stats(y, cs)
                        ln_apply(y, xn, cs, par, pb + 24, pb + 28, m_bc, rstd,
                                 skip_b[g][0])
                    x = xn

                    # ---- FFN + residual + LN2 ----
                    xn2 = sb.tile([128, KC, C], f32r, tag="x", bufs=2)
                    for cs in tiles:
                        Ct = cs[1] - cs[0]
                        hh = sb.tile([128, FKC, Ct], f32r, tag="h", bufs=h_bufs)
                        for m in range(FKC):
                            ps = psp.tile([128, Ct], f32, tag="mm")
                            for k in range(KC):
                                nc.tensor.matmul(
                                    ps[:], f1w[:, k * FH + 128 * m:k * FH + 128 * (m + 1)],
                                    x[:, k, cs[0]:cs[1]],
                                    start=(k == 0), stop=(k == KC - 1))
                            nc.scalar.activation(hh[:, m, :], ps[:], AF.Gelu,
                                                 bias=par[:, pb + 4 + m:pb + 4 + m + 1])
                        y2 = sb.tile([128, KC, C], f32r, tag="y")
                        for m in range(KC):
                            ps = psp.tile([128, Ct], f32, tag="mm")
                            for k in range(FKC):
                                nc.tensor.matmul(
                                    ps[:], f2w[:, k, 128 * m:128 * (m + 1)], hh[:, k, :],
                                    start=(k == 0), stop=(k == FKC - 1))
                            nc.vector.scalar_tensor_tensor(
                                y2[:, m, cs[0]:cs[1]], ps[:],
                                par[:, pb + 20 + m:pb + 20 + m + 1],
                                x[:, m, cs[0]:cs[1]], ALU.add, ALU.add)
                        m_bc, rstd = ln_stats(y2, cs)
                        ln_apply(y2, xn2, cs, par, pb + 32, pb + 36, m_bc, rstd,
                                 skip_b[g][1])
                    x = xn2

                # output proj (pre-scaled by (1-rw)); host adds rw*tgt
                opw = sb.tile([128, KC * H], f32r, tag="io", bufs=1)
                nc.sync.dma_start(opw[:], dram[f"io{g}"].ap()[:, KC * H:2 * KC * H])
                for cs in tiles:
                    for m in range(KC):
                        ps = psp.tile([128, cs[1] - cs[0]], f32, tag="mm")
                        for k in range(KC):
                            nc.tensor.matmul(
                                ps[:],
                                opw[:, k * H + 128 * m:k * H + 128 * (m + 1)],
                                x[:, k, cs[0]:cs[1]],
                                start=(k == 0), stop=(k == KC - 1))
                        ot = sb.tile([128, cs[1] - cs[0]], f32, tag="o", bufs=1)
                        nc.vector.tensor_scalar(
                            ot[:], ps[:], par[:, _P_OPB + m:_P_OPB + m + 1],
                            None, ALU.add)
                        nc.sync.dma_start(
                            dram[f"out{g}"].ap()[:, m * C + cs[0]:m * C + cs[1]], ot[:])

    nc.compile()
    return nc


_CACHE = {}


def _get_program(C0, C1, skip_b, repeat=1):
    key = (C0, C1, skip_b, repeat)
    if key not in _CACHE:
        _CACHE[key] = _build_program(C0, C1, skip_b, repeat)
    return _CACHE[key]


def _prep_gen_weights(i, g_ipw, g_ipb, g_qkv_w, g_qkv_b, g_ao_w, g_ao_b,
                      g_ln1g, g_ln1b, g_ln2g, g_ln2b, g_f1w, g_f1b, g_f2w,
                      g_f2b, g_opw, g_opb, g_rw):
    wa, ba = [], []
    for l in range(L):
        _wq, _wk, wv = np.split(g_qkv_w[i, l], 3, axis=0)
        _bq, _bk, bv = np.split(g_qkv_b[i, l], 3)
        wa.append((g_ao_w[i, l] @ wv).T)                 # [K=H, M=H]
        ba.append(g_ao_b[i, l] + bv @ g_ao_w[i, l].T)
    rw = float(g_rw[i])
    io = np.concatenate([_sb_pack(_round_f32r(g_ipw[i].T)),
                         _sb_pack(_round_f32r((1.0 - rw) * g_opw[i].T))], axis=1)
    waP = np.stack([_sb_pack(_round_f32r(wa[l])) for l in range(L)])
    f1P = np.stack([_sb_pack(_round_f32r(g_f1w[i, l].T)) for l in range(L)])
    f2P = np.stack([_sb_pack(_round_f32r(g_f2w[i, l].T)) for l in range(L)])

    par = np.zeros((128, 128), np.float32)
    par[:, _P_IPB:_P_IPB + KC] = _pack_pcol(g_ipb[i])
    for l in range(L):
        pb = _P_LAYER + 40 * l
        par[:, pb:pb + 4] = _pack_pcol(ba[l])
        par[:, pb + 4:pb + 20] = _pack_pcol(g_f1b[i, l])
        par[:, pb + 20:pb + 24] = _pack_pcol(g_f2b[i, l])
        par[:, pb + 24:pb + 28] = _pack_pcol(g_ln1g[i, l])
        par[:, pb + 28:pb + 32] = _pack_pcol(g_ln1b[i, l])
        par[:, pb + 32:pb + 36] = _pack_pcol(g_ln2g[i, l])
        par[:, pb + 36:pb + 40] = _pack_pcol(g_ln2b[i, l])
    par[:, _P_OPB:_P_OPB + KC] = _pack_pcol((1.0 - rw) * g_opb[i])

    skip = (bool(np.all(g_ln1b[i] == 0.0)), bool(np.all(g_ln2b[i] == 0.0)))
    return {"io": io, "wa": waP, "f1": f1P, "f2": f2P, "par": par}, skip, rw


def _prepare(inputs, repeat=1):
    """Host-side prep. Returns (nc, in_maps, assemble) where assemble(results)
    builds the final outputs."""
    image = np.asarray(inputs["image_features"], np.float32)
    text = np.asarray(inputs["text_features"], np.float32)
    mt = np.asarray(inputs["missing_type"])

    idx1 = np.nonzero(mt == 1)[0]      # gen0 (img -> text) fills text
    idx2 = np.nonzero(mt == 2)[0]      # gen1 (text -> img) fills img
    idx3 = np.nonzero(mt == 3)[0]

    gw = {k: np.asarray(v) for k, v in inputs.items() if k.startswith("g_")}
    w0, skip0, rw0 = _prep_gen_weights(0, **gw)
    w1, skip1, rw1 = _prep_gen_weights(1, **gw)

    # prior MLP on host (tiny)
    pe = np.asarray(inputs["prior_emb"], np.float64)
    t = pe @ np.asarray(inputs["prior_w1"], np.float64).T + np.asarray(inputs["prior_b1"], np.float64)
    t = 0.5 * t * (1.0 + np.vectorize(math.erf)(t / math.sqrt(2.0)))
    prior = (t @ np.asarray(inputs["prior_w2"], np.float64).T
             + np.asarray(inputs["prior_b2"], np.float64)).astype(np.float32)
    p_img, p_text = prior[0, :H], prior[0, H:]

    imgT = np.ascontiguousarray(image.T)
    textT = np.ascontiguousarray(text.T)

    def shard_cols(Tsrc, Ttgt, idx):
        n_pc = max(1, -(-len(idx) // N_CORES))
        C = max(256, -(-n_pc // 64) * 64)
        pad = np.zeros(N_CORES * C, np.int64)
        pad[:len(idx)] = idx
        pad = pad.reshape(N_CORES, C)
        return C, [_sb_pack(_round_f32r(Tsrc[:, pad[c]])) for c in range(N_CORES)], \
            [_sb_pack(_round_f32r(Ttgt[:, pad[c]])) for c in range(N_CORES)]

    C0, src0, tgt0 = shard_cols(imgT, textT, idx1)
    C1, src1, tgt1 = shard_cols(textT, imgT, idx2)

    nc = _get_program(C0, C1, (skip0, skip1), repeat)

    ones = np.ones((128, 128), np.float32)
    in_maps = []
    for c in range(N_CORES):
        in_maps.append({
            "src0": src0[c], "tgt0": tgt0[c], "src1": src1[c], "tgt1": tgt1[c],
            "io0": w0["io"], "wa0": w0["wa"], "f10": w0["f1"], "f20": w0["f2"],
            "par0": w0["par"],
            "io1": w1["io"], "wa1": w1["wa"], "f11": w1["f1"], "f21": w1["f2"],
            "par1": w1["par"],
            "ones": ones,
        })

    def assemble(results):
        def gather_out(name, C, idx, rw, full):
            cols = [results[c][name].reshape(128, KC, C).transpose(1, 0, 2).reshape(H, C)
                    for c in range(N_CORES)]
            allc = np.concatenate(cols, axis=1)[:, :len(idx)]
            return rw * full[idx] + allc.T

        enhanced_text = text.copy()
        if len(idx1):
            enhanced_text[idx1] = gather_out("out0", C0, idx1, rw0, text)
        enhanced_img = image.copy()
        if len(idx2):
            enhanced_img[idx2] = gather_out("out1", C1, idx2, rw1, image)
        if len(idx3):
            enhanced_img[idx3] = p_img
            enhanced_text[idx3] = p_text
        return enhanced_img, enhanced_text

    return nc, in_maps, assemble


def kernel(**inputs):
    nc, in_maps, assemble = _prepare(inputs)
    res = run_bass_kernel_spmd(nc, in_maps, list(range(N_CORES)))
    return assemble(res.results)



# revision 11
# speedup vs baseline: 10.5682x; 10.5682x over previous
"""Trainium2 Bass kernel for nn_ModalGenerator (MoE-routed cross-modal generator).

Strategy (v3):
  - seq_len==1 => attention collapses to v = tgt @ wv.T; fold wv/ao_w into one
    512x512 matrix per layer (host-side) and (1-rw) into the output projection.
  - MoE routing on host: gather missing_type==1 columns (gen0) and ==2 (gen1);
    missing_type==3 rows use the tiny host-computed prior MLP.
  - Generator-split sharding: cores 0-3 run generator 0 on 1/4 of its columns
    each, cores 4-7 run generator 1. Halves per-core weight DMA vs
    data-parallel; zero collectives (host gathers).
  - bf16 activations + bf16 attention/in/out weights; FFN matmuls (2/3 of all
    FLOPs) run in fp8-e4m3 DoubleRow mode (K=256 per pass, 2x PE throughput).
    FFN weights are scaled x64 into e4m3's normal range; the 1/64 unscale is
    folded into the Gelu activation scale and the f2 residual-add scalar.
  - Layer 0 fuses the input projection into the attention PSUM accumulation.
  - Software pipelining: each stage (attn mm / stats / ffn mm / LN scalar
    chain) is issued for all column tiles before the next stage, so one
    tile's LayerNorm dependency chain hides behind the other tile's matmuls.
  - LayerNorm stats via ones-matmul (broadcast column sums); rstd = pow(var,
    -0.5) in one DVE op (bf16 bit-hack + Newton fallback available); Gelu on
    ScalarE; consolidated (2,C)/(4,C) elementwise instructions.
"""

import math

import numpy as np
import ml_dtypes

import concourse.bacc as bacc
import concourse.mybir as mybir
import concourse.tile as tile
from concourse.bass_utils import run_bass_kernel_spmd

f32 = mybir.dt.float32
bf16 = mybir.dt.bfloat16
f8 = mybir.dt.float8e4
i16 = mybir.dt.int16
AF = mybir.ActivationFunctionType
ALU = mybir.AluOpType
DR = mybir.MatmulPerfMode.DoubleRow

H = 512
L = 3
N_CORES = 8
GCORES = 4               # cores per generator
KC = H // 128            # 4 k-chunks of the hidden dim
FH = 4 * H               # 2048 FFN hidden
FKC = FH // 128          # 16
LN_EPS = 1e-5
MAGIC16 = 0x5F37
W8SCALE = 64.0           # fp8 weight pre-scale

FFN_FP8 = True
POW_RSTD = False         # rstd via ALU pow(-0.5); else bit-hack + Newton

# param pack column layout: [128, 128] f32
_P_IPB = 0               # unused when fused (ipb folded into ba[0])
_P_LAYER = 4             # + 40*l: ba 0..3 | f1b 4..19 | f2b 20..23
#                                 | ln1g 24..27 | ln1b 28..31 | ln2g 32..35 | ln2b 36..39
_P_OPB = 124


def _pack_pcol(vec):
    """[n*128] vector -> [128, n] chunk-column layout."""
    return np.ascontiguousarray(np.asarray(vec, np.float32).reshape(-1, 128).T)


def _sb_pack(wT, dt):
    """[K, M] (K mult of 128) -> [128, (K/128)*M] SBUF chunk-major layout."""
    K, M = wT.shape
    a = np.asarray(wT, np.float32).astype(dt)
    return np.ascontiguousarray(
        a.reshape(K // 128, 128, M).transpose(1, 0, 2).reshape(128, -1))


def _tiles(C):
    ts = []
    c0 = 0
    while c0 < C:
        ts.append((c0, min(C, c0 + 512)))
        c0 += 512
    return ts


def _build_program(C, skips):
    """skips = (zero_bias, unit_gamma, zero_beta) -- data-driven fast paths."""
    zb, ug, zbeta = skips
    f8dt = f8 if FFN_FP8 else bf16
    nc = bacc.Bacc("TRN2", target_bir_lowering=False, debug=False,
                   num_devices=N_CORES)

    dram = {
        "src": nc.dram_tensor("src", [128, KC * C], bf16, kind="ExternalInput"),
        "tgt": nc.dram_tensor("tgt", [128, KC * C], bf16, kind="ExternalInput"),
        "io": nc.dram_tensor("io", [128, 2 * KC * H], bf16, kind="ExternalInput"),
        "wa": nc.dram_tensor("wa", [L, 128, KC * H], bf16, kind="ExternalInput"),
        "f1": nc.dram_tensor("f1", [L, 128, KC * FH], f8dt, kind="ExternalInput"),
        "f2": nc.dram_tensor("f2", [L, 128, FKC * H], f8dt, kind="ExternalInput"),
        "par": nc.dram_tensor("par", [128, 128], f32, kind="ExternalInput"),
        "ones": nc.dram_tensor("ones", [128, 128], bf16, kind="ExternalInput"),
        "out": nc.dram_tensor("out", [128, KC * C], bf16, kind="ExternalOutput"),
    }
    tiles = _tiles(C)
    NT = len(tiles)
    unsc = 1.0 / W8SCALE if FFN_FP8 else 1.0

    with tile.TileContext(nc) as tc:
        with (
            tc.tile_pool(name="sb", bufs=2) as sb,
            tc.tile_pool(name="ps", bufs=2, space="PSUM") as psp,
        ):
            ones = sb.tile([128, 128], bf16, tag="ones", bufs=1)
            nc.sync.dma_start(ones[:], dram["ones"].ap())
            par = sb.tile([128, 128], f32, tag="par", bufs=1)
            nc.sync.dma_start(par[:], dram["par"].ap())
            iow = sb.tile([128, 2, KC, H], bf16, tag="io", bufs=1)
            nc.sync.dma_start(iow[:], dram["io"].ap())
            wa0 = sb.tile([128, KC, H], bf16, tag="wa", bufs=2)
            nc.sync.dma_start(wa0[:], dram["wa"].ap()[0])
            src = sb.tile([128, KC, C], bf16, tag="src", bufs=1)
            nc.sync.dma_start(src[:], dram["src"].ap())
            tgt = sb.tile([128, KC, C], bf16, tag="tgt", bufs=1)
            nc.sync.dma_start(tgt[:], dram["tgt"].ap())

            def stats_stage(y, Ct):
                """ACT square + PE column-sum matmuls -> st psum [s, q]."""
                ysq = sb.tile([128, KC, 512], bf16, tag="ysq", bufs=NT)
                nc.scalar.activation(ysq[:, :, 0:Ct], y[:, :, 0:Ct], AF.Square)
                st = psp.tile([128, 2, 512], f32, tag="st", bufs=2)
                for k in range(KC):
                    nc.tensor.matmul(st[:, 0, 0:Ct], ones[:], y[:, k, 0:Ct],
                                     start=(k == 0), stop=(k == KC - 1))
                for k in range(KC):
                    nc.tensor.matmul(st[:, 1, 0:Ct], ones[:], ysq[:, k, 0:Ct],
                                     start=(k == 0), stop=(k == KC - 1))
                return st

            def ln_chain(y, st, Ct, gcol, bcol, xn, xf8, t):
                """DVE scalar chain + apply: y,st -> xn bf16 [+ xf8 fp8]."""
                c0, c1 = t
                m = sb.tile([128, 512], bf16, tag="m", bufs=NT)
                nc.vector.tensor_scalar(m[:, 0:Ct], st[:, 0, 0:Ct], 1.0 / H,
                                        None, ALU.mult)
                msq = sb.tile([128, 512], bf16, tag="msq", bufs=NT)
                nc.vector.scalar_tensor_tensor(msq[:, 0:Ct], st[:, 0, 0:Ct],
                                               1.0 / H, m[:, 0:Ct],
                                               ALU.mult, ALU.mult)
                zq = sb.tile([128, 512], bf16, tag="zq", bufs=NT)
                nc.vector.tensor_scalar(zq[:, 0:Ct], st[:, 1, 0:Ct], 1.0 / H,
                                        LN_EPS, ALU.mult, ALU.add)
                z = sb.tile([128, 512], bf16, tag="z", bufs=NT)
                nc.vector.tensor_sub(z[:, 0:Ct], zq[:, 0:Ct], msq[:, 0:Ct])
                if POW_RSTD:
                    rstd2 = sb.tile([128, 512], bf16, tag="rstd", bufs=2 * NT)
                    nc.vector.tensor_scalar(rstd2[:, 0:Ct], z[:, 0:Ct], -0.5,
                                            None, ALU.pow)
                else:
                    r = sb.tile([128, 512], bf16, tag="rx", bufs=NT)
                    nc.vector.tensor_scalar(r[:, 0:Ct].bitcast(i16),
                                            z[:, 0:Ct].bitcast(i16), 1, None,
                                            ALU.logical_shift_right)
                    rstd = sb.tile([128, 512], bf16, tag="rstd", bufs=2 * NT)
                    nc.vector.tensor_scalar(rstd[:, 0:Ct].bitcast(i16),
                                            r[:, 0:Ct].bitcast(i16), -1,
                                            MAGIC16, ALU.mult, ALU.add)
                    u = sb.tile([128, 512], bf16, tag="u", bufs=NT)
                    nc.vector.tensor_mul(u[:, 0:Ct], rstd[:, 0:Ct],
                                         rstd[:, 0:Ct])
                    w = sb.tile([128, 512], bf16, tag="w", bufs=NT)
                    nc.vector.scalar_tensor_tensor(w[:, 0:Ct], u[:, 0:Ct],
                                                   -0.5, z[:, 0:Ct],
                                                   ALU.mult, ALU.mult)
                    rstd2 = sb.tile([128, 512], bf16, tag="rstd", bufs=2 * NT)
                    nc.vector.scalar_tensor_tensor(rstd2[:, 0:Ct], w[:, 0:Ct],
                                                   1.5, rstd[:, 0:Ct],
                                                   ALU.add, ALU.mult)
                m4 = m[:, 0:Ct].unsqueeze(1).broadcast_to((128, KC, Ct))
                r4 = rstd2[:, 0:Ct].unsqueeze(1).broadcast_to((128, KC, Ct))
                if ug and zbeta:
                    u1 = sb.tile([128, KC, 512], bf16, tag="u1", bufs=NT)
                    nc.vector.tensor_sub(u1[:, :, 0:Ct], y[:, :, 0:Ct], m4)
                    if xf8 is not None:
                        nc.vector.tensor_mul(xf8[:, :, c0:c1], u1[:, :, 0:Ct],
                                             r4)
                    nc.vector.tensor_mul(xn[:, :, c0:c1], u1[:, :, 0:Ct], r4)
                else:
                    for mm in range(KC):
                        u1 = sb.tile([128, 512], bf16, tag="u1c", bufs=1)
                        nc.vector.tensor_sub(u1[:, 0:Ct], y[:, mm, 0:Ct],
                                             m[:, 0:Ct])
                        u2 = sb.tile([128, 512], bf16, tag="u2c", bufs=1)
                        nc.vector.scalar_tensor_tensor(
                            u2[:, 0:Ct], u1[:, 0:Ct],
                            par[:, gcol + mm:gcol + mm + 1], rstd2[:, 0:Ct],
                            ALU.mult, ALU.mult)
                        nc.vector.tensor_scalar(
                            xn[:, mm, c0:c1], u2[:, 0:Ct],
                            par[:, bcol + mm:bcol + mm + 1], None, ALU.add)
                    if xf8 is not None:
                        nc.scalar.activation(xf8[:, :, c0:c1], xn[:, :, c0:c1],
                                             AF.Copy)

            x = None
            for l in range(L):
                pb = _P_LAYER + 40 * l
                if l == 0:
                    wa = wa0
                else:
                    wa = sb.tile([128, KC, H], bf16, tag="wa", bufs=2)
                    nc.sync.dma_start(wa[:], dram["wa"].ap()[l])
                f1w = sb.tile([128, KC, FH], f8dt, tag="f1", bufs=2)
                nc.sync.dma_start(f1w[:], dram["f1"].ap()[l])
                f2w = sb.tile([128, FKC, H], f8dt, tag="f2", bufs=2)
                nc.sync.dma_start(f2w[:], dram["f2"].ap()[l])

                xn = sb.tile([128, KC, C], bf16, tag="x", bufs=3)
                xn2 = sb.tile([128, KC, C], bf16, tag="x", bufs=3)
                if FFN_FP8:
                    xf8n = sb.tile([128, KC, C], f8, tag="xf8", bufs=2)
                else:
                    xf8n = None

                # ---- stage: attention (v-proj) + residual (+input proj l=0)
                ys = []
                for t in tiles:
                    c0, c1 = t
                    Ct = c1 - c0
                    y = sb.tile([128, KC, 512], bf16, tag="y", bufs=NT)
                    for g in range(2):
                        ps = psp.tile([128, 2, 512], f32, tag="mm")
                        for j in range(2):
                            mi = 2 * g + j
                            if l == 0:
                                for k in range(KC):
                                    nc.tensor.matmul(
                                        ps[:, j, 0:Ct],
                                        iow[:, 0, k, 128 * mi:128 * (mi + 1)],
                                        src[:, k, c0:c1], start=(k == 0),
                                        stop=False)
                            for k in range(KC):
                                nc.tensor.matmul(
                                    ps[:, j, 0:Ct],
                                    wa[:, k, 128 * mi:128 * (mi + 1)],
                                    tgt[:, k, c0:c1],
                                    start=(k == 0 and l != 0),
                                    stop=(k == KC - 1))
                        if l == 0:
                            if zb:
                                nc.scalar.activation(
                                    y[:, 2 * g:2 * g + 2, 0:Ct],
                                    ps[:, :, 0:Ct], AF.Copy)
                            else:
                                for j in range(2):
                                    mi = 2 * g + j
                                    nc.scalar.activation(
                                        y[:, mi, 0:Ct], ps[:, j, 0:Ct],
                                        AF.Copy,
                                        bias=par[:, pb + mi:pb + mi + 1])
                        else:
                            if zb:
                                nc.vector.tensor_add(
                                    y[:, 2 * g:2 * g + 2, 0:Ct],
                                    ps[:, :, 0:Ct],
                                    x[:, 2 * g:2 * g + 2, c0:c1])
                            else:
                                for j in range(2):
                                    mi = 2 * g + j
                                    nc.vector.scalar_tensor_tensor(
                                        y[:, mi, 0:Ct], ps[:, j, 0:Ct],
                                        par[:, pb + mi:pb + mi + 1],
                                        x[:, mi, c0:c1], ALU.add, ALU.add)
                    ys.append(y)

                # ---- stage: ln1 stats, then scalar chains
                sts = [stats_stage(ys[i], tiles[i][1] - tiles[i][0])
                       for i in range(NT)]
                for i, t in enumerate(tiles):
                    ln_chain(ys[i], sts[i], t[1] - t[0], pb + 24, pb + 28,
                             xn, xf8n, t)

                # ---- stage: FFN f1 + gelu
                xin = xf8n if FFN_FP8 else xn
                hhs = []
                for t in tiles:
                    c0, c1 = t
                    Ct = c1 - c0
                    hh = sb.tile([128, FKC, 512], f8dt, tag="h", bufs=NT)
                    for g in range(FKC // 2):
                        ps = psp.tile([128, 2, 512], f32, tag="mm")
                        for j in range(2):
                            mi = 2 * g + j
                            if FFN_FP8:
                                for p in range(KC // 2):
                                    nc.tensor.matmul(
                                        ps[:, j, 0:Ct],
                                        f1w[:, 2 * p:2 * p + 2,
                                            128 * mi:128 * (mi + 1)],
                                        xin[:, 2 * p:2 * p + 2, c0:c1],
                                        start=(p == 0), stop=(p == KC // 2 - 1),
                                        perf_mode=DR)
                            else:
                                for k in range(KC):
                                    nc.tensor.matmul(
                                        ps[:, j, 0:Ct],
                                        f1w[:, k, 128 * mi:128 * (mi + 1)],
                                        xin[:, k, c0:c1],
                                        start=(k == 0), stop=(k == KC - 1))
                        if zb:
                            nc.scalar.activation(hh[:, 2 * g:2 * g + 2, 0:Ct],
                                                 ps[:, :, 0:Ct], AF.Gelu,
                                                 scale=unsc)
                        else:
                            for j in range(2):
                                mi = 2 * g + j
                                nc.scalar.activation(
                                    hh[:, mi, 0:Ct], ps[:, j, 0:Ct], AF.Gelu,
                                    scale=unsc,
                                    bias=par[:, pb + 4 + mi:pb + 4 + mi + 1])
                    hhs.append(hh)

                # ---- stage: FFN f2 + residual
                y2s = []
                for i, t in enumerate(tiles):
                    c0, c1 = t
                    Ct = c1 - c0
                    hh = hhs[i]
                    y2 = sb.tile([128, KC, 512], bf16, tag="y", bufs=NT)
                    for g in range(2):
                        ps = psp.tile([128, 2, 512], f32, tag="mm")
                        for j in range(2):
                            mi = 2 * g + j
                            if FFN_FP8:
                                for p in range(FKC // 2):
                                    nc.tensor.matmul(
                                        ps[:, j, 0:Ct],
                                        f2w[:, 2 * p:2 * p + 2,
                                            128 * mi:128 * (mi + 1)],
                                        hh[:, 2 * p:2 * p + 2, 0:Ct],
                                        start=(p == 0),
                                        stop=(p == FKC // 2 - 1),
                                        perf_mode=DR)
                            else:
                                for k in range(FKC):
                                    nc.tensor.matmul(
                                        ps[:, j, 0:Ct],
                                        f2w[:, k, 128 * mi:128 * (mi + 1)],
                                        hh[:, k, 0:Ct],
                                        start=(k == 0), stop=(k == FKC - 1))
                        if zb:
                            nc.vector.scalar_tensor_tensor(
                                y2[:, 2 * g:2 * g + 2, 0:Ct], ps[:, :, 0:Ct],
                                unsc, xn[:, 2 * g:2 * g + 2, c0:c1],
                                ALU.mult, ALU.add)
                        else:
                            for j in range(2):
                                mi = 2 * g + j
                                t1 = sb.tile([128, 512], bf16, tag="t1",
                                             bufs=1)
                                nc.vector.scalar_tensor_tensor(
                                    t1[:, 0:Ct], ps[:, j, 0:Ct], unsc,
                                    xn[:, mi, c0:c1], ALU.mult, ALU.add)
                                nc.vector.tensor_scalar(
                                    y2[:, mi, 0:Ct], t1[:, 0:Ct],
                                    par[:, pb + 20 + mi:pb + 20 + mi + 1],
                                    None, ALU.add)
                    y2s.append(y2)

                # ---- stage: ln2 stats + chains
                sts2 = [stats_stage(y2s[i], tiles[i][1] - tiles[i][0])
                        for i in range(NT)]
                for i, t in enumerate(tiles):
                    ln_chain(y2s[i], sts2[i], t[1] - t[0], pb + 32, pb + 36,
                             xn2, None, t)
                x = xn2

            # ---- output projection (opw pre-scaled by (1-rw)) ----
            for t in tiles:
                c0, c1 = t
                Ct = c1 - c0
                for g in range(2):
                    ps = psp.tile([128, 2, 512], f32, tag="mm")
                    for j in range(2):
                        mi = 2 * g + j
                        for k in range(KC):
                            nc.tensor.matmul(
                                ps[:, j, 0:Ct],
                                iow[:, 1, k, 128 * mi:128 * (mi + 1)],
                                x[:, k, c0:c1],
                                start=(k == 0), stop=(k == KC - 1))
                    ot = sb.tile([128, 2, 512], bf16, tag="o", bufs=2)
                    if zb:
                        nc.scalar.activation(ot[:, :, 0:Ct], ps[:, :, 0:Ct],
                                             AF.Copy)
                    else:
                        for j in range(2):
                            mi = 2 * g + j
                            nc.scalar.activation(
                                ot[:, j, 0:Ct], ps[:, j, 0:Ct], AF.Copy,
                                bias=par[:, _P_OPB + mi:_P_OPB + mi + 1])
                    for j in range(2):
                        mi = 2 * g + j
                        nc.sync.dma_start(
                            dram["out"].ap()[:, mi * C + c0:mi * C + c1],
                            ot[:, j, 0:Ct])

    nc.compile()
    return nc


_CACHE = {}


def _get_program(C, skips):
    key = (C, skips)
    if key not in _CACHE:
        _CACHE[key] = _build_program(C, skips)
    return _CACHE[key]


def _prep_gen_weights(i, g_ipw, g_ipb, g_qkv_w, g_qkv_b, g_ao_w, g_ao_b,
                      g_ln1g, g_ln1b, g_ln2g, g_ln2b, g_f1w, g_f1b, g_f2w,
                      g_f2b, g_opw, g_opb, g_rw):
    wa, ba = [], []
    for l in range(L):
        _wq, _wk, wv = np.split(g_qkv_w[i, l], 3, axis=0)
        _bq, _bk, bv = np.split(g_qkv_b[i, l], 3)
        wa.append((g_ao_w[i, l] @ wv).T)                 # [K=H, M=H]
        ba.append(g_ao_b[i, l] + bv @ g_ao_w[i, l].T)
    rw = float(g_rw[i])
    ws = W8SCALE if FFN_FP8 else 1.0
    f8np = mybir.dt.np(f8 if FFN_FP8 else bf16)
    io = np.concatenate([_sb_pack(g_ipw[i].T, ml_dtypes.bfloat16),
                         _sb_pack((1.0 - rw) * g_opw[i].T, ml_dtypes.bfloat16)],
                        axis=1)
    waP = np.stack([_sb_pack(wa[l], ml_dtypes.bfloat16) for l in range(L)])
    f1P = np.stack([_sb_pack(ws * g_f1w[i, l].T, f8np) for l in range(L)])
    f2P = np.stack([_sb_pack(ws * g_f2w[i, l].T, f8np) for l in range(L)])

    par = np.zeros((128, 128), np.float32)
    par[:, _P_IPB:_P_IPB + KC] = _pack_pcol(g_ipb[i])
    for l in range(L):
        pb = _P_LAYER + 40 * l
        bal = ba[l] + (g_ipb[i] if l == 0 else 0.0)   # layer-0 fuses ipb
        par[:, pb:pb + 4] = _pack_pcol(bal)
        par[:, pb + 4:pb + 20] = _pack_pcol(g_f1b[i, l])
        par[:, pb + 20:pb + 24] = _pack_pcol(g_f2b[i, l])
        par[:, pb + 24:pb + 28] = _pack_pcol(g_ln1g[i, l])
        par[:, pb + 28:pb + 32] = _pack_pcol(g_ln1b[i, l])
        par[:, pb + 32:pb + 36] = _pack_pcol(g_ln2g[i, l])
        par[:, pb + 36:pb + 40] = _pack_pcol(g_ln2b[i, l])
    par[:, _P_OPB:_P_OPB + KC] = _pack_pcol((1.0 - rw) * g_opb[i])

    zb = bool(np.all(g_ipb[i] == 0) and all(np.all(b == 0) for b in ba)
              and np.all(g_f1b[i] == 0) and np.all(g_f2b[i] == 0)
              and np.all(g_opb[i] == 0))
    ug = bool(np.all(g_ln1g[i] == 1) and np.all(g_ln2g[i] == 1))
    zbeta = bool(np.all(g_ln1b[i] == 0) and np.all(g_ln2b[i] == 0))
    return {"io": io, "wa": waP, "f1": f1P, "f2": f2P, "par": par}, \
        (zb, ug, zbeta), rw


def _prepare(inputs):
    """Host-side prep. Returns (nc, in_maps, assemble)."""
    image = np.asarray(inputs["image_features"], np.float32)
    text = np.asarray(inputs["text_features"], np.float32)
    mt = np.asarray(inputs["missing_type"])

    idx1 = np.nonzero(mt == 1)[0]      # gen0 (img -> text) fills text
    idx2 = np.nonzero(mt == 2)[0]      # gen1 (text -> img) fills img
    idx3 = np.nonzero(mt == 3)[0]

    gw = {k: np.asarray(v) for k, v in inputs.items() if k.startswith("g_")}
    w0, skips0, rw0 = _prep_gen_weights(0, **gw)
    w1, skips1, rw1 = _prep_gen_weights(1, **gw)
    skips = tuple(a and b for a, b in zip(skips0, skips1))

    # prior MLP on host (tiny)
    pe = np.asarray(inputs["prior_emb"], np.float64)
    t = pe @ np.asarray(inputs["prior_w1"], np.float64).T \
        + np.asarray(inputs["prior_b1"], np.float64)
    t = 0.5 * t * (1.0 + np.vectorize(math.erf)(t / math.sqrt(2.0)))
    prior = (t @ np.asarray(inputs["prior_w2"], np.float64).T
             + np.asarray(inputs["prior_b2"], np.float64)).astype(np.float32)
    p_img, p_text = prior[0, :H], prior[0, H:]

    imgT = np.ascontiguousarray(image.T)
    textT = np.ascontiguousarray(text.T)

    n_pc = -(-max(len(idx1), len(idx2), 1) // GCORES)   # per-core columns
    C = max(64, -(-n_pc // 64) * 64)                    # round up to 64

    def shard_cols(Tsrc, Ttgt, idx):
        pad = np.zeros(GCORES * C, np.int64)
        pad[:len(idx)] = idx
        pad = pad.reshape(GCORES, C)
        return [_sb_pack(Tsrc[:, pad[c]], ml_dtypes.bfloat16)
                for c in range(GCORES)], \
            [_sb_pack(Ttgt[:, pad[c]], ml_dtypes.bfloat16)
             for c in range(GCORES)]

    src0, tgt0 = shard_cols(imgT, textT, idx1)
    src1, tgt1 = shard_cols(textT, imgT, idx2)

    nc = _get_program(C, skips)

    ones = np.ones((128, 128), ml_dtypes.bfloat16)
    in_maps = []
    for c in range(N_CORES):
        g = 0 if c < GCORES else 1
        w = w0 if g == 0 else w1
        lc = c % GCORES
        in_maps.append({
            "src": (src0 if g == 0 else src1)[lc],
            "tgt": (tgt0 if g == 0 else tgt1)[lc],
            "io": w["io"], "wa": w["wa"], "f1": w["f1"], "f2": w["f2"],
            "par": w["par"], "ones": ones,
        })

    def assemble(results):
        def gather_out(cores, idx, rw, full):
            cols = [np.asarray(results[c]["out"])
                    .astype(np.float32)
                    .reshape(128, KC, C).transpose(1, 0, 2).reshape(H, C)
                    for c in cores]
            allc = np.concatenate(cols, axis=1)[:, :len(idx)]
            return rw * full[idx] + allc.T

        enhanced_text = text.copy()
        if len(idx1):
            enhanced_text[idx1] = gather_out(range(GCORES), idx1, rw0, text)
        enhanced_img = image.copy()
        if len(idx2):
            enhanced_img[idx2] = gather_out(range(GCORES, N_CORES), idx2,
                                            rw1, image)
        if len(idx3):
            enhanced_img[idx3] = p_img
            enhanced_text[idx3] = p_text
        return enhanced_img, enhanced_text

    return nc, in_maps, assemble


def kernel(**inputs):
    nc, in_maps, assemble = _prepare(inputs)
    res = run_bass_kernel_spmd(nc, in_maps, list(range(N_CORES)))
    return assemble(res.results)


# revision 29
# speedup vs baseline: 17.0549x; 1.6138x over previous
"""Trainium2 Bass kernel for nn_ModalGenerator (MoE-routed cross-modal generator).

Strategy (v3):
  - seq_len==1 => attention collapses to v = tgt @ wv.T; fold wv/ao_w into one
    512x512 matrix per layer (host-side) and (1-rw) into the output projection.
  - MoE routing on host: gather missing_type==1 columns (gen0) and ==2 (gen1);
    missing_type==3 rows use the tiny host-computed prior MLP.
  - Generator-split sharding: cores 0-3 run generator 0 on 1/4 of its columns
    each, cores 4-7 run generator 1. Halves per-core weight DMA vs
    data-parallel; zero collectives (host gathers).
  - bf16 activations + bf16 attention/in/out weights; FFN matmuls (2/3 of all
    FLOPs) run in fp8-e4m3 DoubleRow mode (K=256 per pass, 2x PE throughput).
    FFN weights are scaled x64 into e4m3's normal range; the 1/64 unscale is
    folded into the Gelu activation scale and the f2 residual-add scalar.
  - Layer 0 fuses the input projection into the attention PSUM accumulation.
  - Software pipelining: each stage (attn mm / stats / ffn mm / LN scalar
    chain) is issued for all column tiles before the next stage, so one
    tile's LayerNorm dependency chain hides behind the other tile's matmuls.
  - LayerNorm stats via ones-matmul (broadcast column sums); rstd = pow(var,
    -0.5) in one DVE op (bf16 bit-hack + Newton fallback available); Gelu on
    ScalarE; consolidated (2,C)/(4,C) elementwise instructions.
"""

import math

import numpy as np
import ml_dtypes

import concourse.bacc as bacc
import concourse.mybir as mybir
import concourse.tile as tile
from concourse.bass_utils import run_bass_kernel_spmd

f32 = mybir.dt.float32
bf16 = mybir.dt.bfloat16
f8 = mybir.dt.float8e4
i16 = mybir.dt.int16
AF = mybir.ActivationFunctionType
ALU = mybir.AluOpType
DR = mybir.MatmulPerfMode.DoubleRow

H = 512
L = 3
N_CORES = 8
GCORES = 4               # cores per generator
KC = H // 128            # 4 k-chunks of the hidden dim
FH = 4 * H               # 2048 FFN hidden
FKC = FH // 128          # 16
LN_EPS = 1e-5
MAGIC16 = 0x5F37
W8SCALE = 64.0           # fp8 weight pre-scale

FFN_FP8 = True
POW_RSTD = False         # rstd via ALU pow(-0.5); else bit-hack + Newton

# param pack column layout: [128, 128] f32
_P_IPB = 0               # unused when fused (ipb folded into ba[0])
_P_LAYER = 4             # + 40*l: ba 0..3 | f1b 4..19 | f2b 20..23
#                                 | ln1g 24..27 | ln1b 28..31 | ln2g 32..35 | ln2b 36..39
_P_OPB = 124


def _pack_pcol(vec):
    """[n*128] vector -> [128, n] chunk-column layout."""
    return np.ascontiguousarray(np.asarray(vec, np.float32).reshape(-1, 128).T)


def _sb_pack(wT, dt):
    """[K, M] (K mult of 128) -> [128, (K/128)*M] SBUF chunk-major layout."""
    K, M = wT.shape
    a = np.asarray(wT, np.float32).astype(dt)
    return np.ascontiguousarray(
        a.reshape(K // 128, 128, M).transpose(1, 0, 2).reshape(128, -1))


NT_TARGET = 4            # pipeline depth (equal column tiles per core)


def _tiles(C):
    nt = min(NT_TARGET, max(1, C // 64))
    tile_sz = -(-C // nt // 16) * 16
    ts = []
    c0 = 0
    while c0 < C:
        ts.append((c0, min(C, c0 + tile_sz)))
        c0 += tile_sz
    return ts


def _build_program(C, skips):
    """skips = (zero_bias, unit_gamma, zero_beta) -- data-driven fast paths."""
    zb, ug, zbeta = skips
    f8dt = f8 if FFN_FP8 else bf16
    nc = bacc.Bacc("TRN2", target_bir_lowering=False, debug=False,
                   num_devices=N_CORES)

    dram = {
        "src": nc.dram_tensor("src", [128, KC * C], f8, kind="ExternalInput"),
        "tgt": nc.dram_tensor("tgt", [128, KC * C], f8, kind="ExternalInput"),
        "ip": nc.dram_tensor("ip", [128, KC * H], f8, kind="ExternalInput"),
        "op": nc.dram_tensor("op", [128, KC * H], bf16, kind="ExternalInput"),
        "wa": nc.dram_tensor("wa", [L, 128, KC * H], f8, kind="ExternalInput"),
        "f1": nc.dram_tensor("f1", [L, 128, KC * FH], f8dt, kind="ExternalInput"),
        "f2": nc.dram_tensor("f2", [L, 128, FKC * H], f8dt, kind="ExternalInput"),
        "par": nc.dram_tensor("par", [128, 128], f32, kind="ExternalInput"),
        "ones": nc.dram_tensor("ones", [128, 128], bf16, kind="ExternalInput"),
        "ident": nc.dram_tensor("ident", [128, 128], bf16, kind="ExternalInput"),
        "out": nc.dram_tensor("out", [128, KC * C], bf16, kind="ExternalOutput"),
    }
    tiles = _tiles(C)
    NT = len(tiles)
    PB = min(NT, 3)
    unsc = 1.0 / W8SCALE if FFN_FP8 else 1.0

    with tile.TileContext(nc) as tc:
        with (
            tc.tile_pool(name="sb", bufs=2) as sb,
            tc.tile_pool(name="ps", bufs=2, space="PSUM") as psp,
        ):
            ipw = sb.tile([128, KC, H], f8, tag="ip", bufs=1)
            nc.sync.dma_start(ipw[:], dram["ip"].ap())
            wa0 = sb.tile([128, KC, H], f8, tag="wa", bufs=2)
            nc.sync.dma_start(wa0[:], dram["wa"].ap()[0])
            srcT = sb.tile([128, KC * C], f8, tag="src", bufs=1)
            tgtT = sb.tile([128, KC * C], f8, tag="tgt", bufs=1)
            nc.sync.dma_start(srcT[:, 0:KC * tiles[0][1]],
                              dram["src"].ap()[:, 0:KC * tiles[0][1]])
            nc.sync.dma_start(tgtT[:, 0:KC * tiles[0][1]],
                              dram["tgt"].ap()[:, 0:KC * tiles[0][1]])
            ones = sb.tile([128, 128], bf16, tag="ones", bufs=1)
            nc.sync.dma_start(ones[:], dram["ones"].ap())
            ident = sb.tile([128, 128], bf16, tag="ident", bufs=1)
            nc.sync.dma_start(ident[:], dram["ident"].ap())
            for ti in range(1, NT):
                c0, c1 = tiles[ti]
                nc.sync.dma_start(srcT[:, KC * c0:KC * c1],
                                  dram["src"].ap()[:, KC * c0:KC * c1])
                nc.sync.dma_start(tgtT[:, KC * c0:KC * c1],
                                  dram["tgt"].ap()[:, KC * c0:KC * c1])

            def _tm(flat, ti, p):
                c0, c1 = tiles[ti]
                Ct = c1 - c0
                sl = flat[:, KC * c0 + 2 * p * Ct:KC * c0 + (2 * p + 2) * Ct]
                return sl.rearrange("q (a b) -> q a b", a=2)
            par = sb.tile([128, 128], f32, tag="par", bufs=1)
            nc.sync.dma_start(par[:], dram["par"].ap())

            def stats_stage(y, Ct):
                """ACT square + PE column-sum matmuls -> st psum [s, q]."""
                ysq = sb.tile([128, KC, 512], bf16, tag="ysq", bufs=2)
                nc.vector.tensor_mul(ysq[:, :, 0:Ct], y[:, :, 0:Ct],
                                     y[:, :, 0:Ct])
                st = psp.tile([128, 2, 512], f32, tag="st", bufs=1)
                for k in range(KC):
                    nc.tensor.matmul(st[:, 0, 0:Ct], ones[:], y[:, k, 0:Ct],
                                     start=(k == 0), stop=(k == KC - 1))
                for k in range(KC):
                    nc.tensor.matmul(st[:, 1, 0:Ct], ones[:], ysq[:, k, 0:Ct],
                                     start=(k == 0), stop=(k == KC - 1))
                return st

            def ln_chain(y, st, Ct, gcol, bcol, xn, xf8, t):
                """DVE scalar chain + apply: y,st -> xn bf16 [+ xf8 fp8]."""
                c0, c1 = t
                m = sb.tile([128, 512], bf16, tag="m", bufs=2)
                nc.vector.tensor_scalar(m[:, 0:Ct], st[:, 0, 0:Ct], 1.0 / H,
                                        None, ALU.mult)
                msq = sb.tile([128, 512], bf16, tag="msq", bufs=2)
                nc.vector.scalar_tensor_tensor(msq[:, 0:Ct], st[:, 0, 0:Ct],
                                               1.0 / H, m[:, 0:Ct],
                                               ALU.mult, ALU.mult)
                # z = q/H - m^2; eps dropped: padded all-zero columns stay
                # finite through the bit-hack (r^2 < bf16 max), real columns
                # have var >> eps.
                z = sb.tile([128, 512], bf16, tag="z", bufs=2)
                nc.vector.scalar_tensor_tensor(z[:, 0:Ct], st[:, 1, 0:Ct],
                                               1.0 / H, msq[:, 0:Ct],
                                               ALU.mult, ALU.subtract)
                r = sb.tile([128, 512], bf16, tag="rx", bufs=2)
                nc.vector.tensor_scalar(r[:, 0:Ct].bitcast(i16),
                                        z[:, 0:Ct].bitcast(i16), 1, None,
                                        ALU.logical_shift_right)
                rstd = sb.tile([128, 512], bf16, tag="rstd", bufs=4)
                nc.vector.tensor_scalar(rstd[:, 0:Ct].bitcast(i16),
                                        r[:, 0:Ct].bitcast(i16), -1,
                                        MAGIC16, ALU.mult, ALU.add)
                u = sb.tile([128, 512], bf16, tag="u", bufs=2)
                nc.vector.tensor_mul(u[:, 0:Ct], rstd[:, 0:Ct],
                                     rstd[:, 0:Ct])
                w = sb.tile([128, 512], bf16, tag="w", bufs=2)
                nc.vector.scalar_tensor_tensor(w[:, 0:Ct], u[:, 0:Ct],
                                               -0.5, z[:, 0:Ct],
                                               ALU.mult, ALU.mult)
                rstd2 = sb.tile([128, 512], bf16, tag="rstd", bufs=4)
                nc.vector.scalar_tensor_tensor(rstd2[:, 0:Ct], w[:, 0:Ct],
                                               1.5, rstd[:, 0:Ct],
                                               ALU.add, ALU.mult)
                m4 = m[:, 0:Ct].unsqueeze(1).broadcast_to((128, KC, Ct))
                r4 = rstd2[:, 0:Ct].unsqueeze(1).broadcast_to((128, KC, Ct))
                if ug and zbeta:
                    u1 = sb.tile([128, KC, 512], bf16, tag="u1", bufs=2)
                    nc.vector.tensor_sub(u1[:, :, 0:Ct], y[:, :, 0:Ct], m4)
                    if xf8 is not None:
                        nc.gpsimd.tensor_mul(xf8[:, :, c0:c1],
                                             u1[:, :, 0:Ct], r4)
                        nc.gpsimd.tensor_mul(xn[:, :, c0:c1],
                                             u1[:, :, 0:Ct], r4)
                    else:
                        nc.vector.tensor_mul(xn[:, :, c0:c1],
                                             u1[:, :, 0:Ct], r4)
                else:
                    for mm in range(KC):
                        u1 = sb.tile([128, 512], bf16, tag="u1c", bufs=1)
                        nc.vector.tensor_sub(u1[:, 0:Ct], y[:, mm, 0:Ct],
                                             m[:, 0:Ct])
                        u2 = sb.tile([128, 512], bf16, tag="u2c", bufs=1)
                        nc.vector.scalar_tensor_tensor(
                            u2[:, 0:Ct], u1[:, 0:Ct],
                            par[:, gcol + mm:gcol + mm + 1], rstd2[:, 0:Ct],
                            ALU.mult, ALU.mult)
                        nc.vector.tensor_scalar(
                            xn[:, mm, c0:c1], u2[:, 0:Ct],
                            par[:, bcol + mm:bcol + mm + 1], None, ALU.add)
                    if xf8 is not None:
                        nc.scalar.activation(xf8[:, :, c0:c1], xn[:, :, c0:c1],
                                             AF.Copy)

            x = None
            xn_all, xf8_all = [], []
            for l in range(L):
                xn_a = sb.tile([128, KC, C], bf16, tag="x", bufs=3)
                xn_b = sb.tile([128, KC, C], bf16, tag="x", bufs=3)
                xn_all.append((xn_a, xn_b))
                if FFN_FP8:
                    xf8_t = sb.tile([128, KC, C], f8, tag="xf8", bufs=2)
                    xf8_all.append(xf8_t)
                else:
                    xf8_all.append(None)

            was, f1s, f2s = [wa0], [], []
            for l in range(L):
                if l > 0:
                    wa = sb.tile([128, KC, H], f8, tag="wa", bufs=2)
                    nc.sync.dma_start(wa[:], dram["wa"].ap()[l])
                    was.append(wa)
                f1w = sb.tile([128, KC, FH], f8dt, tag="f1", bufs=2)
                nc.sync.dma_start(f1w[:], dram["f1"].ap()[l])
                f1s.append(f1w)
                f2w = sb.tile([128, FKC, H], f8dt, tag="f2", bufs=2)
                nc.sync.dma_start(f2w[:], dram["f2"].ap()[l])
                f2s.append(f2w)
            opw = sb.tile([128, KC, H], bf16, tag="op", bufs=1)
            nc.sync.dma_start(opw[:], dram["op"].ap())

            ys_all = [[None] * NT for _ in range(L)]
            y2s_all = [[None] * NT for _ in range(L)]
            hh_all = [[None] * NT for _ in range(L)]

            def attn_stage(l, ti):
                pb = _P_LAYER + 40 * l
                wa = was[l]
                xp = xn_all[l - 1][1] if l > 0 else None
                c0, c1 = tiles[ti]
                Ct = c1 - c0
                y = sb.tile([128, KC, 512], bf16, tag="y", bufs=NT)
                for g in range(2):
                    ps = psp.tile([128, 2, 512], f32, tag="mm", bufs=3)
                    for j in range(2):
                        mi = 2 * g + j
                        if l == 0:
                            for p in range(KC // 2):
                                nc.tensor.matmul(
                                    ps[:, j, 0:Ct],
                                    ipw[:, 2 * p:2 * p + 2,
                                        128 * mi:128 * (mi + 1)],
                                    _tm(srcT, ti, p),
                                    start=(p == 0), stop=False, perf_mode=DR)
                        for p in range(KC // 2):
                            nc.tensor.matmul(
                                ps[:, j, 0:Ct],
                                wa[:, 2 * p:2 * p + 2, 128 * mi:128 * (mi + 1)],
                                _tm(tgtT, ti, p),
                                start=(p == 0 and l != 0),
                                stop=(p == KC // 2 - 1 and l == 0),
                                perf_mode=DR)
                        if l != 0:
                            # residual via 64-scaled identity (unscale folds)
                            nc.tensor.matmul(
                                ps[:, j, 0:Ct], ident[:],
                                xp[:, mi, c0:c1],
                                start=False, stop=True)
                    if zb:
                        nc.scalar.activation(y[:, 2 * g:2 * g + 2, 0:Ct],
                                             ps[:, :, 0:Ct], AF.Copy,
                                             scale=unsc)
                    else:
                        for j in range(2):
                            mi = 2 * g + j
                            nc.scalar.activation(
                                y[:, mi, 0:Ct], ps[:, j, 0:Ct], AF.Copy,
                                scale=unsc,
                                bias=par[:, pb + mi:pb + mi + 1])
                ys_all[l][ti] = y

            def ln1_stage(l, ti):
                pb = _P_LAYER + 40 * l
                t = tiles[ti]
                st = stats_stage(ys_all[l][ti], t[1] - t[0])
                ln_chain(ys_all[l][ti], st, t[1] - t[0], pb + 24, pb + 28,
                         xn_all[l][0], xf8_all[l], t)

            def f1_stage(l, ti):
                pb = _P_LAYER + 40 * l
                f1w = f1s[l]
                xin = xf8_all[l] if FFN_FP8 else xn_all[l][0]
                c0, c1 = tiles[ti]
                Ct = c1 - c0
                hh = sb.tile([128, FKC, 512], f8dt, tag="h", bufs=2)
                for g in range(FKC // 2):
                    ps = psp.tile([128, 2, 512], f32, tag="mm", bufs=3)
                    for j in range(2):
                        mi = 2 * g + j
                        if FFN_FP8:
                            for p in range(KC // 2):
                                nc.tensor.matmul(
                                    ps[:, j, 0:Ct],
                                    f1w[:, 2 * p:2 * p + 2,
                                        128 * mi:128 * (mi + 1)],
                                    xin[:, 2 * p:2 * p + 2, c0:c1],
                                    start=(p == 0), stop=(p == KC // 2 - 1),
                                    perf_mode=DR)
                        else:
                            for k in range(KC):
                                nc.tensor.matmul(
                                    ps[:, j, 0:Ct],
                                    f1w[:, k, 128 * mi:128 * (mi + 1)],
                                    xin[:, k, c0:c1],
                                    start=(k == 0), stop=(k == KC - 1))
                    if zb:
                        nc.scalar.activation(hh[:, 2 * g:2 * g + 2, 0:Ct],
                                             ps[:, :, 0:Ct], AF.Gelu,
                                             scale=unsc)
                    else:
                        for j in range(2):
                            mi = 2 * g + j
                            nc.scalar.activation(
                                hh[:, mi, 0:Ct], ps[:, j, 0:Ct], AF.Gelu,
                                scale=unsc,
                                bias=par[:, pb + 4 + mi:pb + 4 + mi + 1])
                hh_all[l][ti] = hh

            def f2_stage(l, ti):
                pb = _P_LAYER + 40 * l
                f2w = f2s[l]
                xn = xn_all[l][0]
                hh = hh_all[l][ti]
                c0, c1 = tiles[ti]
                Ct = c1 - c0
                y2 = sb.tile([128, KC, 512], bf16, tag="y", bufs=NT)
                for g in range(2):
                    ps = psp.tile([128, 2, 512], f32, tag="mm", bufs=3)
                    for j in range(2):
                        mi = 2 * g + j
                        if FFN_FP8:
                            for p in range(FKC // 2):
                                nc.tensor.matmul(
                                    ps[:, j, 0:Ct],
                                    f2w[:, 2 * p:2 * p + 2,
                                        128 * mi:128 * (mi + 1)],
                                    hh[:, 2 * p:2 * p + 2, 0:Ct],
                                    start=(p == 0), stop=False,
                                    perf_mode=DR)
                        else:
                            for k in range(FKC):
                                nc.tensor.matmul(
                                    ps[:, j, 0:Ct],
                                    f2w[:, k, 128 * mi:128 * (mi + 1)],
                                    hh[:, k, 0:Ct],
                                    start=(k == 0), stop=False)
                        nc.tensor.matmul(
                            ps[:, j, 0:Ct], ident[:], xn[:, mi, c0:c1],
                            start=False, stop=True)
                    if zb:
                        nc.scalar.activation(y2[:, 2 * g:2 * g + 2, 0:Ct],
                                             ps[:, :, 0:Ct], AF.Copy,
                                             scale=unsc)
                    else:
                        for j in range(2):
                            mi = 2 * g + j
                            nc.scalar.activation(
                                y2[:, mi, 0:Ct], ps[:, j, 0:Ct], AF.Copy,
                                scale=unsc,
                                bias=par[:, pb + 20 + mi:pb + 20 + mi + 1])
                y2s_all[l][ti] = y2

            def ln2_stage(l, ti):
                pb = _P_LAYER + 40 * l
                t = tiles[ti]
                st2 = stats_stage(y2s_all[l][ti], t[1] - t[0])
                ln_chain(y2s_all[l][ti], st2, t[1] - t[0], pb + 32, pb + 36,
                         xn_all[l][1], None, t)

            def out_stage(l, ti):
                xl = xn_all[L - 1][1]
                c0, c1 = tiles[ti]
                Ct = c1 - c0
                for g in range(2):
                    ps = psp.tile([128, 2, 512], f32, tag="mm", bufs=3)
                    for j in range(2):
                        mi = 2 * g + j
                        for k in range(KC):
                            nc.tensor.matmul(
                                ps[:, j, 0:Ct],
                                opw[:, k, 128 * mi:128 * (mi + 1)],
                                xl[:, k, c0:c1],
                                start=(k == 0), stop=(k == KC - 1))
                    ot = sb.tile([128, 2, 512], bf16, tag="o", bufs=2)
                    if zb:
                        nc.scalar.activation(ot[:, :, 0:Ct], ps[:, :, 0:Ct],
                                             AF.Copy)
                    else:
                        for j in range(2):
                            mi = 2 * g + j
                            nc.scalar.activation(
                                ot[:, j, 0:Ct], ps[:, j, 0:Ct], AF.Copy,
                                bias=par[:, _P_OPB + mi:_P_OPB + mi + 1])
                    for j in range(2):
                        mi = 2 * g + j
                        nc.sync.dma_start(
                            dram["out"].ap()[:, mi * C + c0:mi * C + c1],
                            ot[:, j, 0:Ct])

            # wavefront issue: stage ls of tile t at wave ls + t
            stage_fns = []
            for l in range(L):
                stage_fns += [
                    (attn_stage, l), (ln1_stage, l), (f1_stage, l),
                    (f2_stage, l), (ln2_stage, l),
                ]
            stage_fns.append((out_stage, L - 1))
            NS = len(stage_fns)
            for wave in range(NS + NT - 1):
                for ls in range(NS):
                    ti = wave - ls
                    if 0 <= ti < NT:
                        fn, l = stage_fns[ls]
                        fn(l, ti)

    nc.compile()
    return nc


_CACHE = {}


def _get_program(C, skips):
    key = (C, skips)
    if key not in _CACHE:
        _CACHE[key] = _build_program(C, skips)
    return _CACHE[key]


def _prep_gen_weights(i, g_ipw, g_ipb, g_qkv_w, g_qkv_b, g_ao_w, g_ao_b,
                      g_ln1g, g_ln1b, g_ln2g, g_ln2b, g_f1w, g_f1b, g_f2w,
                      g_f2b, g_opw, g_opb, g_rw):
    wa, ba = [], []
    for l in range(L):
        _wq, _wk, wv = np.split(g_qkv_w[i, l], 3, axis=0)
        _bq, _bk, bv = np.split(g_qkv_b[i, l], 3)
        wa.append((g_ao_w[i, l] @ wv).T)                 # [K=H, M=H]
        ba.append(g_ao_b[i, l] + bv @ g_ao_w[i, l].T)
    rw = float(g_rw[i])
    ws = W8SCALE if FFN_FP8 else 1.0
    f8np = mybir.dt.np(f8 if FFN_FP8 else bf16)
    ipP = _sb_pack(W8SCALE * g_ipw[i].T, mybir.dt.np(f8))
    opP = _sb_pack((1.0 - rw) * g_opw[i].T, ml_dtypes.bfloat16)
    waP = np.stack([_sb_pack(W8SCALE * wa[l], mybir.dt.np(f8))
                    for l in range(L)])
    f1P = np.stack([_sb_pack(ws * g_f1w[i, l].T, f8np) for l in range(L)])
    f2P = np.stack([_sb_pack(ws * g_f2w[i, l].T, f8np) for l in range(L)])

    par = np.zeros((128, 128), np.float32)
    par[:, _P_IPB:_P_IPB + KC] = _pack_pcol(g_ipb[i])
    for l in range(L):
        pb = _P_LAYER + 40 * l
        bal = ba[l] + (g_ipb[i] if l == 0 else 0.0)   # layer-0 fuses ipb
        par[:, pb:pb + 4] = _pack_pcol(bal)
        par[:, pb + 4:pb + 20] = _pack_pcol(g_f1b[i, l])
        par[:, pb + 20:pb + 24] = _pack_pcol(g_f2b[i, l])
        par[:, pb + 24:pb + 28] = _pack_pcol(g_ln1g[i, l])
        par[:, pb + 28:pb + 32] = _pack_pcol(g_ln1b[i, l])
        par[:, pb + 32:pb + 36] = _pack_pcol(g_ln2g[i, l])
        par[:, pb + 36:pb + 40] = _pack_pcol(g_ln2b[i, l])
    par[:, _P_OPB:_P_OPB + KC] = _pack_pcol((1.0 - rw) * g_opb[i])

    zb = bool(np.all(g_ipb[i] == 0) and all(np.all(b == 0) for b in ba)
              and np.all(g_f1b[i] == 0) and np.all(g_f2b[i] == 0)
              and np.all(g_opb[i] == 0))
    ug = bool(np.all(g_ln1g[i] == 1) and np.all(g_ln2g[i] == 1))
    zbeta = bool(np.all(g_ln1b[i] == 0) and np.all(g_ln2b[i] == 0))
    return {"ip": ipP, "op": opP, "wa": waP, "f1": f1P, "f2": f2P,
            "par": par}, (zb, ug, zbeta), rw


def _prepare(inputs):
    """Host-side prep. Returns (nc, in_maps, assemble)."""
    image = np.asarray(inputs["image_features"], np.float32)
    text = np.asarray(inputs["text_features"], np.float32)
    mt = np.asarray(inputs["missing_type"])

    idx1 = np.nonzero(mt == 1)[0]      # gen0 (img -> text) fills text
    idx2 = np.nonzero(mt == 2)[0]      # gen1 (text -> img) fills img
    idx3 = np.nonzero(mt == 3)[0]

    gw = {k: np.asarray(v) for k, v in inputs.items() if k.startswith("g_")}
    w0, skips0, rw0 = _prep_gen_weights(0, **gw)
    w1, skips1, rw1 = _prep_gen_weights(1, **gw)
    skips = tuple(a and b for a, b in zip(skips0, skips1))

    # prior MLP on host (tiny)
    pe = np.asarray(inputs["prior_emb"], np.float64)
    t = pe @ np.asarray(inputs["prior_w1"], np.float64).T \
        + np.asarray(inputs["prior_b1"], np.float64)
    t = 0.5 * t * (1.0 + np.vectorize(math.erf)(t / math.sqrt(2.0)))
    prior = (t @ np.asarray(inputs["prior_w2"], np.float64).T
             + np.asarray(inputs["prior_b2"], np.float64)).astype(np.float32)
    p_img, p_text = prior[0, :H], prior[0, H:]

    imgT = np.ascontiguousarray(image.T)
    textT = np.ascontiguousarray(text.T)

    n_pc = -(-max(len(idx1), len(idx2), 1) // GCORES)   # per-core columns
    C = max(64, -(-n_pc // 64) * 64)                    # round up to 64

    tls = _tiles(C)

    def _pack_tm(M):
        """[H, C] -> tile-major [128, NT*KC*Tt] fp8."""
        a = M.astype(mybir.dt.np(f8)).reshape(KC, 128, C).transpose(1, 0, 2)
        return np.concatenate(
            [np.ascontiguousarray(a[:, :, t0:t1]).reshape(128, -1)
             for t0, t1 in tls], axis=1)

    def shard_cols(Tsrc, Ttgt, idx):
        pad = np.zeros(GCORES * C, np.int64)
        pad[:len(idx)] = idx
        pad = pad.reshape(GCORES, C)
        return [_pack_tm(Tsrc[:, pad[c]]) for c in range(GCORES)], \
            [_pack_tm(Ttgt[:, pad[c]]) for c in range(GCORES)]

    src0, tgt0 = shard_cols(imgT, textT, idx1)
    src1, tgt1 = shard_cols(textT, imgT, idx2)

    nc = _get_program(C, skips)

    ones = np.ones((128, 128), ml_dtypes.bfloat16)
    ident = (np.eye(128, dtype=np.float32) * W8SCALE).astype(ml_dtypes.bfloat16)
    in_maps = []
    for c in range(N_CORES):
        g = 0 if c < GCORES else 1
        w = w0 if g == 0 else w1
        lc = c % GCORES
        in_maps.append({
            "src": (src0 if g == 0 else src1)[lc],
            "tgt": (tgt0 if g == 0 else tgt1)[lc],
            "ip": w["ip"], "op": w["op"], "wa": w["wa"], "f1": w["f1"],
            "f2": w["f2"],
            "par": w["par"], "ones": ones, "ident": ident,
        })

    def assemble(results):
        def gather_out(cores, idx, rw, full):
            cols = [np.asarray(results[c]["out"])
                    .astype(np.float32)
                    .reshape(128, KC, C).transpose(1, 0, 2).reshape(H, C)
                    for c in cores]
            allc = np.concatenate(cols, axis=1)[:, :len(idx)]
            return rw * full[idx] + allc.T

        enhanced_text = text.copy()
        if len(idx1):
            enhanced_text[idx1] = gather_out(range(GCORES), idx1, rw0, text)
        enhanced_img = image.copy()
        if len(idx2):
            enhanced_img[idx2] = gather_out(range(GCORES, N_CORES), idx2,
                                            rw1, image)
        if len(idx3):
            enhanced_img[idx3] = p_img
            enhanced_text[idx3] = p_text
        return enhanced_img, enhanced_text

    return nc, in_maps, assemble


def kernel(**inputs):
    nc, in_maps, assemble = _prepare(inputs)
    res = run_bass_kernel_spmd(nc, in_maps, list(range(N_CORES)))
    return assemble(res.results)


# revision 32
# speedup vs baseline: 18.3432x; 1.0755x over previous
"""Trainium2 Bass kernel for nn_ModalGenerator (MoE-routed cross-modal generator).

Strategy (v3):
  - seq_len==1 => attention collapses to v = tgt @ wv.T; fold wv/ao_w into one
    512x512 matrix per layer (host-side) and (1-rw) into the output projection.
  - MoE routing on host: gather missing_type==1 columns (gen0) and ==2 (gen1);
    missing_type==3 rows use the tiny host-computed prior MLP.
  - Generator-split sharding: cores 0-3 run generator 0 on 1/4 of its columns
    each, cores 4-7 run generator 1. Halves per-core weight DMA vs
    data-parallel; zero collectives (host gathers).
  - bf16 activations + bf16 attention/in/out weights; FFN matmuls (2/3 of all
    FLOPs) run in fp8-e4m3 DoubleRow mode (K=256 per pass, 2x PE throughput).
    FFN weights are scaled x64 into e4m3's normal range; the 1/64 unscale is
    folded into the Gelu activation scale and the f2 residual-add scalar.
  - Layer 0 fuses the input projection into the attention PSUM accumulation.
  - Software pipelining: each stage (attn mm / stats / ffn mm / LN scalar
    chain) is issued for all column tiles before the next stage, so one
    tile's LayerNorm dependency chain hides behind the other tile's matmuls.
  - LayerNorm stats via ones-matmul (broadcast column sums); rstd = pow(var,
    -0.5) in one DVE op (bf16 bit-hack + Newton fallback available); Gelu on
    ScalarE; consolidated (2,C)/(4,C) elementwise instructions.
"""

import math

import numpy as np
import ml_dtypes

import concourse.bacc as bacc
import concourse.mybir as mybir
import concourse.tile as tile
from concourse.bass_utils import run_bass_kernel_spmd

f32 = mybir.dt.float32
bf16 = mybir.dt.bfloat16
f8 = mybir.dt.float8e4
i16 = mybir.dt.int16
AF = mybir.ActivationFunctionType
ALU = mybir.AluOpType
DR = mybir.MatmulPerfMode.DoubleRow

H = 512
L = 3
N_CORES = 8
GCORES = 4               # cores per generator
KC = H // 128            # 4 k-chunks of the hidden dim
FH = 4 * H               # 2048 FFN hidden
FKC = FH // 128          # 16
LN_EPS = 1e-5
MAGIC16 = 0x5F37
W8SCALE = 64.0           # fp8 weight pre-scale

FFN_FP8 = True
POW_RSTD = False         # rstd via ALU pow(-0.5); else bit-hack + Newton

# param pack column layout: [128, 128] f32
_P_IPB = 0               # unused when fused (ipb folded into ba[0])
_P_LAYER = 4             # + 40*l: ba 0..3 | f1b 4..19 | f2b 20..23
#                                 | ln1g 24..27 | ln1b 28..31 | ln2g 32..35 | ln2b 36..39
_P_OPB = 124


def _pack_pcol(vec):
    """[n*128] vector -> [128, n] chunk-column layout."""
    return np.ascontiguousarray(np.asarray(vec, np.float32).reshape(-1, 128).T)


def _sb_pack(wT, dt):
    """[K, M] (K mult of 128) -> [128, (K/128)*M] SBUF chunk-major layout."""
    K, M = wT.shape
    a = np.asarray(wT, np.float32).astype(dt)
    return np.ascontiguousarray(
        a.reshape(K // 128, 128, M).transpose(1, 0, 2).reshape(128, -1))


NT_TARGET = 4            # pipeline depth (equal column tiles per core)


def _tiles(C):
    nt = min(NT_TARGET, max(1, C // 64))
    tile_sz = -(-C // nt // 16) * 16
    ts = []
    c0 = 0
    while c0 < C:
        ts.append((c0, min(C, c0 + tile_sz)))
        c0 += tile_sz
    return ts


def _build_program(C, skips):
    """skips = (zero_bias, unit_gamma, zero_beta) -- data-driven fast paths."""
    zb, ug, zbeta = skips
    f8dt = f8 if FFN_FP8 else bf16
    nc = bacc.Bacc("TRN2", target_bir_lowering=False, debug=False,
                   num_devices=N_CORES)

    dram = {
        "src": nc.dram_tensor("src", [128, KC * C], f8, kind="ExternalInput"),
        "tgt": nc.dram_tensor("tgt", [128, KC * C], f8, kind="ExternalInput"),
        "ip": nc.dram_tensor("ip", [128, KC * H], f8, kind="ExternalInput"),
        "op": nc.dram_tensor("op", [128, KC * H], bf16, kind="ExternalInput"),
        "wa": nc.dram_tensor("wa", [L, 128, KC * H], f8, kind="ExternalInput"),
        "f1": nc.dram_tensor("f1", [L, 128, KC * FH], f8dt, kind="ExternalInput"),
        "f2": nc.dram_tensor("f2", [L, 128, FKC * H], f8dt, kind="ExternalInput"),
        "par": nc.dram_tensor("par", [128, 128], f32, kind="ExternalInput"),
        "ones": nc.dram_tensor("ones", [128, 128], bf16, kind="ExternalInput"),
        "ident": nc.dram_tensor("ident", [128, 128], bf16, kind="ExternalInput"),
        "out": nc.dram_tensor("out", [128, KC * C], bf16, kind="ExternalOutput"),
    }
    tiles = _tiles(C)
    NT = len(tiles)
    PB = min(NT, 3)
    unsc = 1.0 / W8SCALE if FFN_FP8 else 1.0

    with tile.TileContext(nc) as tc:
        with (
            tc.tile_pool(name="sb", bufs=2) as sb,
            tc.tile_pool(name="ps", bufs=2, space="PSUM") as psp,
        ):
            ipw = sb.tile([128, KC, H], f8, tag="ip", bufs=1)
            nc.sync.dma_start(ipw[:], dram["ip"].ap())
            wa0 = sb.tile([128, KC, H], f8, tag="wa", bufs=2)
            nc.sync.dma_start(wa0[:], dram["wa"].ap()[0])
            srcT = sb.tile([128, KC * C], f8, tag="src", bufs=1)
            tgtT = sb.tile([128, KC * C], f8, tag="tgt", bufs=1)
            nc.sync.dma_start(srcT[:, 0:KC * tiles[0][1]],
                              dram["src"].ap()[:, 0:KC * tiles[0][1]])
            nc.sync.dma_start(tgtT[:, 0:KC * tiles[0][1]],
                              dram["tgt"].ap()[:, 0:KC * tiles[0][1]])
            ones = sb.tile([128, 128], bf16, tag="ones", bufs=1)
            nc.sync.dma_start(ones[:], dram["ones"].ap())
            ident = sb.tile([128, 128], bf16, tag="ident", bufs=1)
            nc.sync.dma_start(ident[:], dram["ident"].ap())
            for ti in range(1, NT):
                c0, c1 = tiles[ti]
                nc.sync.dma_start(srcT[:, KC * c0:KC * c1],
                                  dram["src"].ap()[:, KC * c0:KC * c1])
                nc.sync.dma_start(tgtT[:, KC * c0:KC * c1],
                                  dram["tgt"].ap()[:, KC * c0:KC * c1])

            def _tm(flat, ti, p):
                c0, c1 = tiles[ti]
                Ct = c1 - c0
                sl = flat[:, KC * c0 + 2 * p * Ct:KC * c0 + (2 * p + 2) * Ct]
                return sl.rearrange("q (a b) -> q a b", a=2)
            par = sb.tile([128, 128], f32, tag="par", bufs=1)
            nc.sync.dma_start(par[:], dram["par"].ap())

            def stats_stage(y, Ct):
                """ACT square + PE column-sum matmuls -> st psum [s, q]."""
                ysq = sb.tile([128, KC, 512], bf16, tag="ysq", bufs=2)
                nc.vector.tensor_mul(ysq[:, :, 0:Ct], y[:, :, 0:Ct],
                                     y[:, :, 0:Ct])
                st = psp.tile([128, 2, 512], f32, tag="st", bufs=1)
                for k in range(KC):
                    nc.tensor.matmul(st[:, 0, 0:Ct], ones[:], y[:, k, 0:Ct],
                                     start=(k == 0), stop=(k == KC - 1))
                for k in range(KC):
                    nc.tensor.matmul(st[:, 1, 0:Ct], ones[:], ysq[:, k, 0:Ct],
                                     start=(k == 0), stop=(k == KC - 1))
                return st

            def ln_chain(y, st, Ct, gcol, bcol, xn, xf8, t):
                """DVE scalar chain + apply: y,st -> xn bf16 [+ xf8 fp8]."""
                c0, c1 = t
                m = sb.tile([128, 512], bf16, tag="m", bufs=2)
                nc.vector.tensor_scalar(m[:, 0:Ct], st[:, 0, 0:Ct], 1.0 / H,
                                        None, ALU.mult)
                msq = sb.tile([128, 512], bf16, tag="msq", bufs=2)
                nc.vector.scalar_tensor_tensor(msq[:, 0:Ct], st[:, 0, 0:Ct],
                                               1.0 / H, m[:, 0:Ct],
                                               ALU.mult, ALU.mult)
                # z = q/H - m^2; eps dropped: padded all-zero columns stay
                # finite through the bit-hack (r^2 < bf16 max), real columns
                # have var >> eps.
                z = sb.tile([128, 512], bf16, tag="z", bufs=2)
                nc.vector.scalar_tensor_tensor(z[:, 0:Ct], st[:, 1, 0:Ct],
                                               1.0 / H, msq[:, 0:Ct],
                                               ALU.mult, ALU.subtract)
                r = sb.tile([128, 512], bf16, tag="rx", bufs=2)
                nc.vector.tensor_scalar(r[:, 0:Ct].bitcast(i16),
                                        z[:, 0:Ct].bitcast(i16), 1, None,
                                        ALU.logical_shift_right)
                rstd = sb.tile([128, 512], bf16, tag="rstd", bufs=4)
                nc.vector.tensor_scalar(rstd[:, 0:Ct].bitcast(i16),
                                        r[:, 0:Ct].bitcast(i16), -1,
                                        MAGIC16, ALU.mult, ALU.add)
                u = sb.tile([128, 512], bf16, tag="u", bufs=2)
                nc.vector.tensor_mul(u[:, 0:Ct], rstd[:, 0:Ct],
                                     rstd[:, 0:Ct])
                w = sb.tile([128, 512], bf16, tag="w", bufs=2)
                nc.vector.scalar_tensor_tensor(w[:, 0:Ct], u[:, 0:Ct],
                                               -0.5, z[:, 0:Ct],
                                               ALU.mult, ALU.mult)
                rstd2 = sb.tile([128, 512], bf16, tag="rstd", bufs=4)
                nc.vector.scalar_tensor_tensor(rstd2[:, 0:Ct], w[:, 0:Ct],
                                               1.5, rstd[:, 0:Ct],
                                               ALU.add, ALU.mult)
                m4 = m[:, 0:Ct].unsqueeze(1).broadcast_to((128, KC, Ct))
                r4 = rstd2[:, 0:Ct].unsqueeze(1).broadcast_to((128, KC, Ct))
                if ug and zbeta:
                    u1 = sb.tile([128, KC, 512], bf16, tag="u1", bufs=2)
                    nc.vector.tensor_sub(u1[:, :, 0:Ct], y[:, :, 0:Ct], m4)
                    if xf8 is not None:
                        nc.vector.tensor_mul(xf8[:, :, c0:c1],
                                             u1[:, :, 0:Ct], r4)
                        nc.gpsimd.tensor_mul(xn[:, :, c0:c1],
                                             u1[:, :, 0:Ct], r4)
                    else:
                        nc.vector.tensor_mul(xn[:, :, c0:c1],
                                             u1[:, :, 0:Ct], r4)
                else:
                    for mm in range(KC):
                        u1 = sb.tile([128, 512], bf16, tag="u1c", bufs=1)
                        nc.vector.tensor_sub(u1[:, 0:Ct], y[:, mm, 0:Ct],
                                             m[:, 0:Ct])
                        u2 = sb.tile([128, 512], bf16, tag="u2c", bufs=1)
                        nc.vector.scalar_tensor_tensor(
                            u2[:, 0:Ct], u1[:, 0:Ct],
                            par[:, gcol + mm:gcol + mm + 1], rstd2[:, 0:Ct],
                            ALU.mult, ALU.mult)
                        nc.vector.tensor_scalar(
                            xn[:, mm, c0:c1], u2[:, 0:Ct],
                            par[:, bcol + mm:bcol + mm + 1], None, ALU.add)
                    if xf8 is not None:
                        nc.scalar.activation(xf8[:, :, c0:c1], xn[:, :, c0:c1],
                                             AF.Copy)

            x = None
            xn_all, xf8_all = [], []
            for l in range(L):
                xn_a = sb.tile([128, KC, C], bf16, tag="x", bufs=3)
                xn_b = sb.tile([128, KC, C], bf16, tag="x", bufs=3)
                xn_all.append((xn_a, xn_b))
                if FFN_FP8:
                    xf8_t = sb.tile([128, KC, C], f8, tag="xf8", bufs=2)
                    xf8_all.append(xf8_t)
                else:
                    xf8_all.append(None)

            was, f1s, f2s = [wa0], [], []
            for l in range(L):
                if l > 0:
                    wa = sb.tile([128, KC, H], f8, tag="wa", bufs=2)
                    nc.sync.dma_start(wa[:], dram["wa"].ap()[l])
                    was.append(wa)
                f1w = sb.tile([128, KC, FH], f8dt, tag="f1", bufs=2)
                nc.sync.dma_start(f1w[:], dram["f1"].ap()[l])
                f1s.append(f1w)
                f2w = sb.tile([128, FKC, H], f8dt, tag="f2", bufs=2)
                nc.sync.dma_start(f2w[:], dram["f2"].ap()[l])
                f2s.append(f2w)
            opw = sb.tile([128, KC, H], bf16, tag="op", bufs=1)
            nc.sync.dma_start(opw[:], dram["op"].ap())

            ys_all = [[None] * NT for _ in range(L)]
            y2s_all = [[None] * NT for _ in range(L)]
            hh_all = [[None] * NT for _ in range(L)]

            def attn_stage(l, ti):
                pb = _P_LAYER + 40 * l
                wa = was[l]
                xp = xn_all[l - 1][1] if l > 0 else None
                c0, c1 = tiles[ti]
                Ct = c1 - c0
                y = sb.tile([128, KC, 512], bf16, tag="y", bufs=NT)
                for g in range(2):
                    ps = psp.tile([128, 2, 512], f32, tag="mm", bufs=3)
                    for j in range(2):
                        mi = 2 * g + j
                        if l == 0:
                            for p in range(KC // 2):
                                nc.tensor.matmul(
                                    ps[:, j, 0:Ct],
                                    ipw[:, 2 * p:2 * p + 2,
                                        128 * mi:128 * (mi + 1)],
                                    _tm(srcT, ti, p),
                                    start=(p == 0), stop=False, perf_mode=DR)
                        for p in range(KC // 2):
                            nc.tensor.matmul(
                                ps[:, j, 0:Ct],
                                wa[:, 2 * p:2 * p + 2, 128 * mi:128 * (mi + 1)],
                                _tm(tgtT, ti, p),
                                start=(p == 0 and l != 0),
                                stop=(p == KC // 2 - 1 and l == 0),
                                perf_mode=DR)
                        if l != 0:
                            # residual via 64-scaled identity (unscale folds)
                            nc.tensor.matmul(
                                ps[:, j, 0:Ct], ident[:],
                                xp[:, mi, c0:c1],
                                start=False, stop=True)
                    if zb:
                        nc.scalar.activation(y[:, 2 * g:2 * g + 2, 0:Ct],
                                             ps[:, :, 0:Ct], AF.Copy,
                                             scale=unsc)
                    else:
                        for j in range(2):
                            mi = 2 * g + j
                            nc.scalar.activation(
                                y[:, mi, 0:Ct], ps[:, j, 0:Ct], AF.Copy,
                                scale=unsc,
                                bias=par[:, pb + mi:pb + mi + 1])
                ys_all[l][ti] = y

            def ln1_stage(l, ti):
                pb = _P_LAYER + 40 * l
                t = tiles[ti]
                st = stats_stage(ys_all[l][ti], t[1] - t[0])
                ln_chain(ys_all[l][ti], st, t[1] - t[0], pb + 24, pb + 28,
                         xn_all[l][0], xf8_all[l], t)

            def f1_stage(l, ti):
                pb = _P_LAYER + 40 * l
                f1w = f1s[l]
                xin = xf8_all[l] if FFN_FP8 else xn_all[l][0]
                c0, c1 = tiles[ti]
                Ct = c1 - c0
                hh = sb.tile([128, FKC, 512], f8dt, tag="h", bufs=2)
                for g in range(FKC // 2):
                    ps = psp.tile([128, 2, 512], f32, tag="mm", bufs=3)
                    for j in range(2):
                        mi = 2 * g + j
                        if FFN_FP8:
                            for p in range(KC // 2):
                                nc.tensor.matmul(
                                    ps[:, j, 0:Ct],
                                    f1w[:, 2 * p:2 * p + 2,
                                        128 * mi:128 * (mi + 1)],
                                    xin[:, 2 * p:2 * p + 2, c0:c1],
                                    start=(p == 0), stop=(p == KC // 2 - 1),
                                    perf_mode=DR)
                        else:
                            for k in range(KC):
                                nc.tensor.matmul(
                                    ps[:, j, 0:Ct],
                                    f1w[:, k, 128 * mi:128 * (mi + 1)],
                                    xin[:, k, c0:c1],
                                    start=(k == 0), stop=(k == KC - 1))
                    if zb:
                        nc.scalar.activation(hh[:, 2 * g:2 * g + 2, 0:Ct],
                                             ps[:, :, 0:Ct], AF.Gelu,
                                             scale=unsc)
                    else:
                        for j in range(2):
                            mi = 2 * g + j
                            nc.scalar.activation(
                                hh[:, mi, 0:Ct], ps[:, j, 0:Ct], AF.Gelu,
                                scale=unsc,
                                bias=par[:, pb + 4 + mi:pb + 4 + mi + 1])
                hh_all[l][ti] = hh

            def f2_stage(l, ti):
                pb = _P_LAYER + 40 * l
                f2w = f2s[l]
                xn = xn_all[l][0]
                hh = hh_all[l][ti]
                c0, c1 = tiles[ti]
                Ct = c1 - c0
                y2 = sb.tile([128, KC, 512], bf16, tag="y", bufs=NT)
                for g in range(2):
                    ps = psp.tile([128, 2, 512], f32, tag="mm", bufs=3)
                    for j in range(2):
                        mi = 2 * g + j
                        if FFN_FP8:
                            for p in range(FKC // 2):
                                nc.tensor.matmul(
                                    ps[:, j, 0:Ct],
                                    f2w[:, 2 * p:2 * p + 2,
                                        128 * mi:128 * (mi + 1)],
                                    hh[:, 2 * p:2 * p + 2, 0:Ct],
                                    start=(p == 0), stop=False,
                                    perf_mode=DR)
                        else:
                            for k in range(FKC):
                                nc.tensor.matmul(
                                    ps[:, j, 0:Ct],
                                    f2w[:, k, 128 * mi:128 * (mi + 1)],
                                    hh[:, k, 0:Ct],
                                    start=(k == 0), stop=False)
                        nc.tensor.matmul(
                            ps[:, j, 0:Ct], ident[:], xn[:, mi, c0:c1],
                            start=False, stop=True)
                    if zb:
                        nc.scalar.activation(y2[:, 2 * g:2 * g + 2, 0:Ct],
                                             ps[:, :, 0:Ct], AF.Copy,
                                             scale=unsc)
                    else:
                        for j in range(2):
                            mi = 2 * g + j
                            nc.scalar.activation(
                                y2[:, mi, 0:Ct], ps[:, j, 0:Ct], AF.Copy,
                                scale=unsc,
                                bias=par[:, pb + 20 + mi:pb + 20 + mi + 1])
                y2s_all[l][ti] = y2

            def ln2_stage(l, ti):
                pb = _P_LAYER + 40 * l
                t = tiles[ti]
                st2 = stats_stage(y2s_all[l][ti], t[1] - t[0])
                ln_chain(y2s_all[l][ti], st2, t[1] - t[0], pb + 32, pb + 36,
                         xn_all[l][1], None, t)

            def out_stage(l, ti):
                xl = xn_all[L - 1][1]
                c0, c1 = tiles[ti]
                Ct = c1 - c0
                for g in range(2):
                    ps = psp.tile([128, 2, 512], f32, tag="mm", bufs=3)
                    for j in range(2):
                        mi = 2 * g + j
                        for k in range(KC):
                            nc.tensor.matmul(
                                ps[:, j, 0:Ct],
                                opw[:, k, 128 * mi:128 * (mi + 1)],
                                xl[:, k, c0:c1],
                                start=(k == 0), stop=(k == KC - 1))
                    ot = sb.tile([128, 2, 512], bf16, tag="o", bufs=2)
                    if zb:
                        nc.scalar.activation(ot[:, :, 0:Ct], ps[:, :, 0:Ct],
                                             AF.Copy)
                    else:
                        for j in range(2):
                            mi = 2 * g + j
                            nc.scalar.activation(
                                ot[:, j, 0:Ct], ps[:, j, 0:Ct], AF.Copy,
                                bias=par[:, _P_OPB + mi:_P_OPB + mi + 1])
                    for j in range(2):
                        mi = 2 * g + j
                        nc.sync.dma_start(
                            dram["out"].ap()[:, mi * C + c0:mi * C + c1],
                            ot[:, j, 0:Ct])

            # wavefront issue: stage ls of tile t at wave ls + t
            stage_fns = []
            for l in range(L):
                stage_fns += [
                    (attn_stage, l), (ln1_stage, l), (f1_stage, l),
                    (f2_stage, l), (ln2_stage, l),
                ]
            stage_fns.append((out_stage, L - 1))
            NS = len(stage_fns)
            for wave in range(NS + NT - 1):
                for ls in range(NS - 1, -1, -1):
                    ti = wave - ls
                    if 0 <= ti < NT:
                        fn, l = stage_fns[ls]
                        fn(l, ti)

    nc.compile()
    return nc


_CACHE = {}


def _get_program(C, skips):
    key = (C, skips)
    if key not in _CACHE:
        _CACHE[key] = _build_program(C, skips)
    return _CACHE[key]


def _prep_gen_weights(i, g_ipw, g_ipb, g_qkv_w, g_qkv_b, g_ao_w, g_ao_b,
                      g_ln1g, g_ln1b, g_ln2g, g_ln2b, g_f1w, g_f1b, g_f2w,
                      g_f2b, g_opw, g_opb, g_rw):
    wa, ba = [], []
    for l in range(L):
        _wq, _wk, wv = np.split(g_qkv_w[i, l], 3, axis=0)
        _bq, _bk, bv = np.split(g_qkv_b[i, l], 3)
        wa.append((g_ao_w[i, l] @ wv).T)                 # [K=H, M=H]
        ba.append(g_ao_b[i, l] + bv @ g_ao_w[i, l].T)
    rw = float(g_rw[i])
    ws = W8SCALE if FFN_FP8 else 1.0
    f8np = mybir.dt.np(f8 if FFN_FP8 else bf16)
    ipP = _sb_pack(W8SCALE * g_ipw[i].T, mybir.dt.np(f8))
    opP = _sb_pack((1.0 - rw) * g_opw[i].T, ml_dtypes.bfloat16)
    waP = np.stack([_sb_pack(W8SCALE * wa[l], mybir.dt.np(f8))
                    for l in range(L)])
    f1P = np.stack([_sb_pack(ws * g_f1w[i, l].T, f8np) for l in range(L)])
    f2P = np.stack([_sb_pack(ws * g_f2w[i, l].T, f8np) for l in range(L)])

    par = np.zeros((128, 128), np.float32)
    par[:, _P_IPB:_P_IPB + KC] = _pack_pcol(g_ipb[i])
    for l in range(L):
        pb = _P_LAYER + 40 * l
        bal = ba[l] + (g_ipb[i] if l == 0 else 0.0)   # layer-0 fuses ipb
        par[:, pb:pb + 4] = _pack_pcol(bal)
        par[:, pb + 4:pb + 20] = _pack_pcol(g_f1b[i, l])
        par[:, pb + 20:pb + 24] = _pack_pcol(g_f2b[i, l])
        par[:, pb + 24:pb + 28] = _pack_pcol(g_ln1g[i, l])
        par[:, pb + 28:pb + 32] = _pack_pcol(g_ln1b[i, l])
        par[:, pb + 32:pb + 36] = _pack_pcol(g_ln2g[i, l])
        par[:, pb + 36:pb + 40] = _pack_pcol(g_ln2b[i, l])
    par[:, _P_OPB:_P_OPB + KC] = _pack_pcol((1.0 - rw) * g_opb[i])

    zb = bool(np.all(g_ipb[i] == 0) and all(np.all(b == 0) for b in ba)
              and np.all(g_f1b[i] == 0) and np.all(g_f2b[i] == 0)
              and np.all(g_opb[i] == 0))
    ug = bool(np.all(g_ln1g[i] == 1) and np.all(g_ln2g[i] == 1))
    zbeta = bool(np.all(g_ln1b[i] == 0) and np.all(g_ln2b[i] == 0))
    return {"ip": ipP, "op": opP, "wa": waP, "f1": f1P, "f2": f2P,
            "par": par}, (zb, ug, zbeta), rw


def _prepare(inputs):
    """Host-side prep. Returns (nc, in_maps, assemble)."""
    image = np.asarray(inputs["image_features"], np.float32)
    text = np.asarray(inputs["text_features"], np.float32)
    mt = np.asarray(inputs["missing_type"])

    idx1 = np.nonzero(mt == 1)[0]      # gen0 (img -> text) fills text
    idx2 = np.nonzero(mt == 2)[0]      # gen1 (text -> img) fills img
    idx3 = np.nonzero(mt == 3)[0]

    gw = {k: np.asarray(v) for k, v in inputs.items() if k.startswith("g_")}
    w0, skips0, rw0 = _prep_gen_weights(0, **gw)
    w1, skips1, rw1 = _prep_gen_weights(1, **gw)
    skips = tuple(a and b for a, b in zip(skips0, skips1))

    # prior MLP on host (tiny)
    pe = np.asarray(inputs["prior_emb"], np.float64)
    t = pe @ np.asarray(inputs["prior_w1"], np.float64).T \
        + np.asarray(inputs["prior_b1"], np.float64)
    t = 0.5 * t * (1.0 + np.vectorize(math.erf)(t / math.sqrt(2.0)))
    prior = (t @ np.asarray(inputs["prior_w2"], np.float64).T
             + np.asarray(inputs["prior_b2"], np.float64)).astype(np.float32)
    p_img, p_text = prior[0, :H], prior[0, H:]

    imgT = np.ascontiguousarray(image.T)
    textT = np.ascontiguousarray(text.T)

    n_pc = -(-max(len(idx1), len(idx2), 1) // GCORES)   # per-core columns
    C = max(64, -(-n_pc // 64) * 64)                    # round up to 64

    tls = _tiles(C)

    def _pack_tm(M):
        """[H, C] -> tile-major [128, NT*KC*Tt] fp8."""
        a = M.astype(mybir.dt.np(f8)).reshape(KC, 128, C).transpose(1, 0, 2)
        return np.concatenate(
            [np.ascontiguousarray(a[:, :, t0:t1]).reshape(128, -1)
             for t0, t1 in tls], axis=1)

    def shard_cols(Tsrc, Ttgt, idx):
        pad = np.zeros(GCORES * C, np.int64)
        pad[:len(idx)] = idx
        pad = pad.reshape(GCORES, C)
        return [_pack_tm(Tsrc[:, pad[c]]) for c in range(GCORES)], \
            [_pack_tm(Ttgt[:, pad[c]]) for c in range(GCORES)]

    src0, tgt0 = shard_cols(imgT, textT, idx1)
    src1, tgt1 = shard_cols(textT, imgT, idx2)

    nc = _get_program(C, skips)

    ones = np.ones((128, 128), ml_dtypes.bfloat16)
    ident = (np.eye(128, dtype=np.float32) * W8SCALE).astype(ml_dtypes.bfloat16)
    in_maps = []
    for c in range(N_CORES):
        g = 0 if c < GCORES else 1
        w = w0 if g == 0 else w1
        lc = c % GCORES
        in_maps.append({
            "src": (src0 if g == 0 else src1)[lc],
            "tgt": (tgt0 if g == 0 else tgt1)[lc],
            "ip": w["ip"], "op": w["op"], "wa": w["wa"], "f1": w["f1"],
            "f2": w["f2"],
            "par": w["par"], "ones": ones, "ident": ident,
        })

    def assemble(results):
        def gather_out(cores, idx, rw, full):
            cols = [np.asarray(results[c]["out"])
                    .astype(np.float32)
                    .reshape(128, KC, C).transpose(1, 0, 2).reshape(H, C)
                    for c in cores]
            allc = np.concatenate(cols, axis=1)[:, :len(idx)]
            return rw * full[idx] + allc.T

        enhanced_text = text.copy()
        if len(idx1):
            enhanced_text[idx1] = gather_out(range(GCORES), idx1, rw0, text)
        enhanced_img = image.copy()
        if len(idx2):
            enhanced_img[idx2] = gather_out(range(GCORES, N_CORES), idx2,
                                            rw1, image)
        if len(idx3):
            enhanced_img[idx3] = p_img
            enhanced_text[idx3] = p_text
        return enhanced_img, enhanced_text

    return nc, in_maps, assemble


def kernel(**inputs):
    nc, in_maps, assemble = _prepare(inputs)
    res = run_bass_kernel_spmd(nc, in_maps, list(range(N_CORES)))
    return assemble(res.results)


# revision 36
# speedup vs baseline: 18.3791x; 1.0020x over previous
"""Trainium2 Bass kernel for nn_ModalGenerator (MoE-routed cross-modal generator).

Strategy (v3):
  - seq_len==1 => attention collapses to v = tgt @ wv.T; fold wv/ao_w into one
    512x512 matrix per layer (host-side) and (1-rw) into the output projection.
  - MoE routing on host: gather missing_type==1 columns (gen0) and ==2 (gen1);
    missing_type==3 rows use the tiny host-computed prior MLP.
  - Generator-split sharding: cores 0-3 run generator 0 on 1/4 of its columns
    each, cores 4-7 run generator 1. Halves per-core weight DMA vs
    data-parallel; zero collectives (host gathers).
  - bf16 activations + bf16 attention/in/out weights; FFN matmuls (2/3 of all
    FLOPs) run in fp8-e4m3 DoubleRow mode (K=256 per pass, 2x PE throughput).
    FFN weights are scaled x64 into e4m3's normal range; the 1/64 unscale is
    folded into the Gelu activation scale and the f2 residual-add scalar.
  - Layer 0 fuses the input projection into the attention PSUM accumulation.
  - Software pipelining: each stage (attn mm / stats / ffn mm / LN scalar
    chain) is issued for all column tiles before the next stage, so one
    tile's LayerNorm dependency chain hides behind the other tile's matmuls.
  - LayerNorm stats via ones-matmul (broadcast column sums); rstd = pow(var,
    -0.5) in one DVE op (bf16 bit-hack + Newton fallback available); Gelu on
    ScalarE; consolidated (2,C)/(4,C) elementwise instructions.
"""

import math

import numpy as np
import ml_dtypes

import concourse.bacc as bacc
import concourse.mybir as mybir
import concourse.tile as tile
from concourse.bass_utils import run_bass_kernel_spmd

f32 = mybir.dt.float32
bf16 = mybir.dt.bfloat16
f8 = mybir.dt.float8e4
i16 = mybir.dt.int16
AF = mybir.ActivationFunctionType
ALU = mybir.AluOpType
DR = mybir.MatmulPerfMode.DoubleRow

H = 512
L = 3
N_CORES = 8
GCORES = 4               # cores per generator
KC = H // 128            # 4 k-chunks of the hidden dim
FH = 4 * H               # 2048 FFN hidden
FKC = FH // 128          # 16
LN_EPS = 1e-5
MAGIC16 = 0x5F37
W8SCALE = 64.0           # fp8 weight pre-scale

FFN_FP8 = True
POW_RSTD = False         # rstd via ALU pow(-0.5); else bit-hack + Newton

# param pack column layout: [128, 128] f32
_P_IPB = 0               # unused when fused (ipb folded into ba[0])
_P_LAYER = 4             # + 40*l: ba 0..3 | f1b 4..19 | f2b 20..23
#                                 | ln1g 24..27 | ln1b 28..31 | ln2g 32..35 | ln2b 36..39
_P_OPB = 124


def _pack_pcol(vec):
    """[n*128] vector -> [128, n] chunk-column layout."""
    return np.ascontiguousarray(np.asarray(vec, np.float32).reshape(-1, 128).T)


def _sb_pack(wT, dt):
    """[K, M] (K mult of 128) -> [128, (K/128)*M] SBUF chunk-major layout."""
    K, M = wT.shape
    a = np.asarray(wT, np.float32).astype(dt)
    return np.ascontiguousarray(
        a.reshape(K // 128, 128, M).transpose(1, 0, 2).reshape(128, -1))


NT_TARGET = 4            # pipeline depth (equal column tiles per core)


def _tiles(C):
    nt = min(NT_TARGET, max(1, C // 64))
    base = C // nt // 16 * 16
    sizes = [base] * nt
    extra = C - base * nt
    i = 0
    while extra > 0:
        sizes[i] += min(16, extra)
        extra -= 16
        i = (i + 1) % nt
    # stagger: bigger tiles first (pipeline fill), smallest last (drain)
    if nt >= 2 and min(sizes) - 16 * (nt - 1) >= 64:
        sizes = [s + 16 * (nt - 1 - 2 * i) for i, s in enumerate(sizes)]
    sizes.sort(reverse=True)
    ts = []
    c0 = 0
    for s in sizes:
        if s > 0:
            ts.append((c0, c0 + s))
            c0 += s
    assert c0 == C
    return ts


def _build_program(C, skips):
    """skips = (zero_bias, unit_gamma, zero_beta) -- data-driven fast paths."""
    zb, ug, zbeta = skips
    f8dt = f8 if FFN_FP8 else bf16
    nc = bacc.Bacc("TRN2", target_bir_lowering=False, debug=False,
                   num_devices=N_CORES)

    dram = {
        "src": nc.dram_tensor("src", [128, KC * C], f8, kind="ExternalInput"),
        "tgt": nc.dram_tensor("tgt", [128, KC * C], f8, kind="ExternalInput"),
        "ip": nc.dram_tensor("ip", [128, KC * H], f8, kind="ExternalInput"),
        "op": nc.dram_tensor("op", [128, KC * H], bf16, kind="ExternalInput"),
        "wa": nc.dram_tensor("wa", [L, 128, KC * H], f8, kind="ExternalInput"),
        "f1": nc.dram_tensor("f1", [L, 128, KC * FH], f8dt, kind="ExternalInput"),
        "f2": nc.dram_tensor("f2", [L, 128, FKC * H], f8dt, kind="ExternalInput"),
        "par": nc.dram_tensor("par", [128, 128], f32, kind="ExternalInput"),
        "ones": nc.dram_tensor("ones", [128, 128], bf16, kind="ExternalInput"),
        "ident": nc.dram_tensor("ident", [128, 128], bf16, kind="ExternalInput"),
        "out": nc.dram_tensor("out", [128, KC * C], bf16, kind="ExternalOutput"),
    }
    tiles = _tiles(C)
    NT = len(tiles)
    PB = min(NT, 3)
    unsc = 1.0 / W8SCALE if FFN_FP8 else 1.0

    with tile.TileContext(nc) as tc:
        with (
            tc.tile_pool(name="sb", bufs=2) as sb,
            tc.tile_pool(name="ps", bufs=2, space="PSUM") as psp,
        ):
            ipw = sb.tile([128, KC, H], f8, tag="ip", bufs=1)
            nc.sync.dma_start(ipw[:], dram["ip"].ap())
            wa0 = sb.tile([128, KC, H], f8, tag="wa", bufs=2)
            nc.sync.dma_start(wa0[:], dram["wa"].ap()[0])
            srcT = sb.tile([128, KC * C], f8, tag="src", bufs=1)
            tgtT = sb.tile([128, KC * C], f8, tag="tgt", bufs=1)
            nc.sync.dma_start(srcT[:, 0:KC * tiles[0][1]],
                              dram["src"].ap()[:, 0:KC * tiles[0][1]])
            nc.sync.dma_start(tgtT[:, 0:KC * tiles[0][1]],
                              dram["tgt"].ap()[:, 0:KC * tiles[0][1]])
            ones = sb.tile([128, 128], bf16, tag="ones", bufs=1)
            nc.sync.dma_start(ones[:], dram["ones"].ap())
            ident = sb.tile([128, 128], bf16, tag="ident", bufs=1)
            nc.sync.dma_start(ident[:], dram["ident"].ap())
            for ti in range(1, NT):
                c0, c1 = tiles[ti]
                nc.sync.dma_start(srcT[:, KC * c0:KC * c1],
                                  dram["src"].ap()[:, KC * c0:KC * c1])
                nc.sync.dma_start(tgtT[:, KC * c0:KC * c1],
                                  dram["tgt"].ap()[:, KC * c0:KC * c1])

            def _tm(flat, ti, p):
                c0, c1 = tiles[ti]
                Ct = c1 - c0
                sl = flat[:, KC * c0 + 2 * p * Ct:KC * c0 + (2 * p + 2) * Ct]
                return sl.rearrange("q (a b) -> q a b", a=2)
            par = sb.tile([128, 128], f32, tag="par", bufs=1)
            nc.sync.dma_start(par[:], dram["par"].ap())

            def stats_stage(y, Ct):
                """ACT square + PE column-sum matmuls -> st psum [s, q]."""
                ysq = sb.tile([128, KC, 512], bf16, tag="ysq", bufs=2)
                nc.vector.tensor_mul(ysq[:, :, 0:Ct], y[:, :, 0:Ct],
                                     y[:, :, 0:Ct])
                st = psp.tile([128, 2, 512], f32, tag="st", bufs=1)
                for k in range(KC):
                    nc.tensor.matmul(st[:, 0, 0:Ct], ones[:], y[:, k, 0:Ct],
                                     start=(k == 0), stop=(k == KC - 1))
                for k in range(KC):
                    nc.tensor.matmul(st[:, 1, 0:Ct], ones[:], ysq[:, k, 0:Ct],
                                     start=(k == 0), stop=(k == KC - 1))
                return st

            def ln_chain(y, st, Ct, gcol, bcol, xn, xf8, t):
                """DVE scalar chain + apply: y,st -> xn bf16 [+ xf8 fp8]."""
                c0, c1 = t
                m = sb.tile([128, 512], bf16, tag="m", bufs=2)
                nc.vector.tensor_scalar(m[:, 0:Ct], st[:, 0, 0:Ct], 1.0 / H,
                                        None, ALU.mult)
                msq = sb.tile([128, 512], bf16, tag="msq", bufs=2)
                nc.vector.scalar_tensor_tensor(msq[:, 0:Ct], st[:, 0, 0:Ct],
                                               1.0 / H, m[:, 0:Ct],
                                               ALU.mult, ALU.mult)
                # z = q/H - m^2; eps dropped: padded all-zero columns stay
                # finite through the bit-hack (r^2 < bf16 max), real columns
                # have var >> eps.
                z = sb.tile([128, 512], bf16, tag="z", bufs=2)
                nc.vector.scalar_tensor_tensor(z[:, 0:Ct], st[:, 1, 0:Ct],
                                               1.0 / H, msq[:, 0:Ct],
                                               ALU.mult, ALU.subtract)
                r = sb.tile([128, 512], bf16, tag="rx", bufs=2)
                nc.vector.tensor_scalar(r[:, 0:Ct].bitcast(i16),
                                        z[:, 0:Ct].bitcast(i16), 1, None,
                                        ALU.logical_shift_right)
                rstd = sb.tile([128, 512], bf16, tag="rstd", bufs=4)
                nc.vector.tensor_scalar(rstd[:, 0:Ct].bitcast(i16),
                                        r[:, 0:Ct].bitcast(i16), -1,
                                        MAGIC16, ALU.mult, ALU.add)
                u = sb.tile([128, 512], bf16, tag="u", bufs=2)
                nc.vector.tensor_mul(u[:, 0:Ct], rstd[:, 0:Ct],
                                     rstd[:, 0:Ct])
                w = sb.tile([128, 512], bf16, tag="w", bufs=2)
                nc.vector.scalar_tensor_tensor(w[:, 0:Ct], u[:, 0:Ct],
                                               -0.5, z[:, 0:Ct],
                                               ALU.mult, ALU.mult)
                rstd2 = sb.tile([128, 512], bf16, tag="rstd", bufs=4)
                nc.vector.scalar_tensor_tensor(rstd2[:, 0:Ct], w[:, 0:Ct],
                                               1.5, rstd[:, 0:Ct],
                                               ALU.add, ALU.mult)
                m4 = m[:, 0:Ct].unsqueeze(1).broadcast_to((128, KC, Ct))
                r4 = rstd2[:, 0:Ct].unsqueeze(1).broadcast_to((128, KC, Ct))
                if ug and zbeta:
                    u1 = sb.tile([128, KC, 512], bf16, tag="u1", bufs=2)
                    nc.vector.tensor_sub(u1[:, :, 0:Ct], y[:, :, 0:Ct], m4)
                    if xf8 is not None:
                        nc.vector.tensor_mul(xf8[:, :, c0:c1],
                                             u1[:, :, 0:Ct], r4)
                        nc.gpsimd.tensor_mul(xn[:, :, c0:c1],
                                             u1[:, :, 0:Ct], r4)
                    else:
                        nc.vector.tensor_mul(xn[:, :, c0:c1],
                                             u1[:, :, 0:Ct], r4)
                else:
                    for mm in range(KC):
                        u1 = sb.tile([128, 512], bf16, tag="u1c", bufs=1)
                        nc.vector.tensor_sub(u1[:, 0:Ct], y[:, mm, 0:Ct],
                                             m[:, 0:Ct])
                        u2 = sb.tile([128, 512], bf16, tag="u2c", bufs=1)
                        nc.vector.scalar_tensor_tensor(
                            u2[:, 0:Ct], u1[:, 0:Ct],
                            par[:, gcol + mm:gcol + mm + 1], rstd2[:, 0:Ct],
                            ALU.mult, ALU.mult)
                        nc.vector.tensor_scalar(
                            xn[:, mm, c0:c1], u2[:, 0:Ct],
                            par[:, bcol + mm:bcol + mm + 1], None, ALU.add)
                    if xf8 is not None:
                        nc.scalar.activation(xf8[:, :, c0:c1], xn[:, :, c0:c1],
                                             AF.Copy)

            x = None
            xn_all, xf8_all = [], []
            for l in range(L):
                xn_a = sb.tile([128, KC, C], bf16, tag="x", bufs=3)
                xn_b = sb.tile([128, KC, C], bf16, tag="x", bufs=3)
                xn_all.append((xn_a, xn_b))
                if FFN_FP8:
                    xf8_t = sb.tile([128, KC, C], f8, tag="xf8", bufs=2)
                    xf8_all.append(xf8_t)
                else:
                    xf8_all.append(None)

            was, f1s, f2s = [wa0], [], []
            for l in range(L):
                if l > 0:
                    wa = sb.tile([128, KC, H], f8, tag="wa", bufs=2)
                    nc.sync.dma_start(wa[:], dram["wa"].ap()[l])
                    was.append(wa)
                f1w = sb.tile([128, KC, FH], f8dt, tag="f1", bufs=2)
                nc.sync.dma_start(f1w[:], dram["f1"].ap()[l])
                f1s.append(f1w)
                f2w = sb.tile([128, FKC, H], f8dt, tag="f2", bufs=2)
                nc.sync.dma_start(f2w[:], dram["f2"].ap()[l])
                f2s.append(f2w)
            opw = sb.tile([128, KC, H], bf16, tag="op", bufs=1)
            nc.sync.dma_start(opw[:], dram["op"].ap())

            ys_all = [[None] * NT for _ in range(L)]
            y2s_all = [[None] * NT for _ in range(L)]
            hh_all = [[None] * NT for _ in range(L)]

            def attn_stage(l, ti):
                pb = _P_LAYER + 40 * l
                wa = was[l]
                xp = xn_all[l - 1][1] if l > 0 else None
                c0, c1 = tiles[ti]
                Ct = c1 - c0
                y = sb.tile([128, KC, 512], bf16, tag="y", bufs=NT)
                for g in range(2):
                    ps = psp.tile([128, 2, 512], f32, tag="mm", bufs=3)
                    for j in range(2):
                        mi = 2 * g + j
                        if l == 0:
                            for p in range(KC // 2):
                                nc.tensor.matmul(
                                    ps[:, j, 0:Ct],
                                    ipw[:, 2 * p:2 * p + 2,
                                        128 * mi:128 * (mi + 1)],
                                    _tm(srcT, ti, p),
                                    start=(p == 0), stop=False, perf_mode=DR)
                        for p in range(KC // 2):
                            nc.tensor.matmul(
                                ps[:, j, 0:Ct],
                                wa[:, 2 * p:2 * p + 2, 128 * mi:128 * (mi + 1)],
                                _tm(tgtT, ti, p),
                                start=(p == 0 and l != 0),
                                stop=(p == KC // 2 - 1 and l == 0),
                                perf_mode=DR)
                        if l != 0:
                            # residual via 64-scaled identity (unscale folds)
                            nc.tensor.matmul(
                                ps[:, j, 0:Ct], ident[:],
                                xp[:, mi, c0:c1],
                                start=False, stop=True)
                    if zb:
                        nc.scalar.activation(y[:, 2 * g:2 * g + 2, 0:Ct],
                                             ps[:, :, 0:Ct], AF.Copy,
                                             scale=unsc)
                    else:
                        for j in range(2):
                            mi = 2 * g + j
                            nc.scalar.activation(
                                y[:, mi, 0:Ct], ps[:, j, 0:Ct], AF.Copy,
                                scale=unsc,
                                bias=par[:, pb + mi:pb + mi + 1])
                ys_all[l][ti] = y

            def ln1_stage(l, ti):
                pb = _P_LAYER + 40 * l
                t = tiles[ti]
                st = stats_stage(ys_all[l][ti], t[1] - t[0])
                ln_chain(ys_all[l][ti], st, t[1] - t[0], pb + 24, pb + 28,
                         xn_all[l][0], xf8_all[l], t)

            def f1_stage(l, ti):
                pb = _P_LAYER + 40 * l
                f1w = f1s[l]
                xin = xf8_all[l] if FFN_FP8 else xn_all[l][0]
                c0, c1 = tiles[ti]
                Ct = c1 - c0
                hh = sb.tile([128, FKC, 512], f8dt, tag="h", bufs=2)
                for g in range(FKC // 2):
                    ps = psp.tile([128, 2, 512], f32, tag="mm", bufs=3)
                    for j in range(2):
                        mi = 2 * g + j
                        if FFN_FP8:
                            for p in range(KC // 2):
                                nc.tensor.matmul(
                                    ps[:, j, 0:Ct],
                                    f1w[:, 2 * p:2 * p + 2,
                                        128 * mi:128 * (mi + 1)],
                                    xin[:, 2 * p:2 * p + 2, c0:c1],
                                    start=(p == 0), stop=(p == KC // 2 - 1),
                                    perf_mode=DR)
                        else:
                            for k in range(KC):
                                nc.tensor.matmul(
                                    ps[:, j, 0:Ct],
                                    f1w[:, k, 128 * mi:128 * (mi + 1)],
                                    xin[:, k, c0:c1],
                                    start=(k == 0), stop=(k == KC - 1))
                    if zb:
                        nc.scalar.activation(hh[:, 2 * g:2 * g + 2, 0:Ct],
                                             ps[:, :, 0:Ct], AF.Gelu,
                                             scale=unsc)
                    else:
                        for j in range(2):
                            mi = 2 * g + j
                            nc.scalar.activation(
                                hh[:, mi, 0:Ct], ps[:, j, 0:Ct], AF.Gelu,
                                scale=unsc,
                                bias=par[:, pb + 4 + mi:pb + 4 + mi + 1])
                hh_all[l][ti] = hh

            def f2_stage(l, ti):
                pb = _P_LAYER + 40 * l
                f2w = f2s[l]
                xn = xn_all[l][0]
                hh = hh_all[l][ti]
                c0, c1 = tiles[ti]
                Ct = c1 - c0
                y2 = sb.tile([128, KC, 512], bf16, tag="y", bufs=NT)
                for g in range(2):
                    ps = psp.tile([128, 2, 512], f32, tag="mm", bufs=3)
                    for j in range(2):
                        mi = 2 * g + j
                        if FFN_FP8:
                            for p in range(FKC // 2):
                                nc.tensor.matmul(
                                    ps[:, j, 0:Ct],
                                    f2w[:, 2 * p:2 * p + 2,
                                        128 * mi:128 * (mi + 1)],
                                    hh[:, 2 * p:2 * p + 2, 0:Ct],
                                    start=(p == 0), stop=False,
                                    perf_mode=DR)
                        else:
                            for k in range(FKC):
                                nc.tensor.matmul(
                                    ps[:, j, 0:Ct],
                                    f2w[:, k, 128 * mi:128 * (mi + 1)],
                                    hh[:, k, 0:Ct],
                                    start=(k == 0), stop=False)
                        nc.tensor.matmul(
                            ps[:, j, 0:Ct], ident[:], xn[:, mi, c0:c1],
                            start=False, stop=True)
                    if zb:
                        nc.scalar.activation(y2[:, 2 * g:2 * g + 2, 0:Ct],
                                             ps[:, :, 0:Ct], AF.Copy,
                                             scale=unsc)
                    else:
                        for j in range(2):
                            mi = 2 * g + j
                            nc.scalar.activation(
                                y2[:, mi, 0:Ct], ps[:, j, 0:Ct], AF.Copy,
                                scale=unsc,
                                bias=par[:, pb + 20 + mi:pb + 20 + mi + 1])
                y2s_all[l][ti] = y2

            def ln2_stage(l, ti):
                pb = _P_LAYER + 40 * l
                t = tiles[ti]
                st2 = stats_stage(y2s_all[l][ti], t[1] - t[0])
                ln_chain(y2s_all[l][ti], st2, t[1] - t[0], pb + 32, pb + 36,
                         xn_all[l][1], None, t)

            def out_stage(l, ti):
                xl = xn_all[L - 1][1]
                c0, c1 = tiles[ti]
                Ct = c1 - c0
                for g in range(2):
                    ps = psp.tile([128, 2, 512], f32, tag="mm", bufs=3)
                    for j in range(2):
                        mi = 2 * g + j
                        for k in range(KC):
                            nc.tensor.matmul(
                                ps[:, j, 0:Ct],
                                opw[:, k, 128 * mi:128 * (mi + 1)],
                                xl[:, k, c0:c1],
                                start=(k == 0), stop=(k == KC - 1))
                    ot = sb.tile([128, 2, 512], bf16, tag="o", bufs=2)
                    if zb:
                        nc.scalar.activation(ot[:, :, 0:Ct], ps[:, :, 0:Ct],
                                             AF.Copy)
                    else:
                        for j in range(2):
                            mi = 2 * g + j
                            nc.scalar.activation(
                                ot[:, j, 0:Ct], ps[:, j, 0:Ct], AF.Copy,
                                bias=par[:, _P_OPB + mi:_P_OPB + mi + 1])
                    for j in range(2):
                        mi = 2 * g + j
                        nc.sync.dma_start(
                            dram["out"].ap()[:, mi * C + c0:mi * C + c1],
                            ot[:, j, 0:Ct])

            # wavefront issue: stage ls of tile t at wave ls + t
            stage_fns = []
            for l in range(L):
                stage_fns += [
                    (attn_stage, l), (ln1_stage, l), (f1_stage, l),
                    (f2_stage, l), (ln2_stage, l),
                ]
            stage_fns.append((out_stage, L - 1))
            NS = len(stage_fns)
            for wave in range(NS + NT - 1):
                for ls in range(NS - 1, -1, -1):
                    ti = wave - ls
                    if 0 <= ti < NT:
                        fn, l = stage_fns[ls]
                        fn(l, ti)

    nc.compile()
    return nc


_CACHE = {}


def _get_program(C, skips):
    key = (C, skips)
    if key not in _CACHE:
        _CACHE[key] = _build_program(C, skips)
    return _CACHE[key]


def _prep_gen_weights(i, g_ipw, g_ipb, g_qkv_w, g_qkv_b, g_ao_w, g_ao_b,
                      g_ln1g, g_ln1b, g_ln2g, g_ln2b, g_f1w, g_f1b, g_f2w,
                      g_f2b, g_opw, g_opb, g_rw):
    wa, ba = [], []
    for l in range(L):
        _wq, _wk, wv = np.split(g_qkv_w[i, l], 3, axis=0)
        _bq, _bk, bv = np.split(g_qkv_b[i, l], 3)
        wa.append((g_ao_w[i, l] @ wv).T)                 # [K=H, M=H]
        ba.append(g_ao_b[i, l] + bv @ g_ao_w[i, l].T)
    rw = float(g_rw[i])
    ws = W8SCALE if FFN_FP8 else 1.0
    f8np = mybir.dt.np(f8 if FFN_FP8 else bf16)
    ipP = _sb_pack(W8SCALE * g_ipw[i].T, mybir.dt.np(f8))
    opP = _sb_pack((1.0 - rw) * g_opw[i].T, ml_dtypes.bfloat16)
    waP = np.stack([_sb_pack(W8SCALE * wa[l], mybir.dt.np(f8))
                    for l in range(L)])
    f1P = np.stack([_sb_pack(ws * g_f1w[i, l].T, f8np) for l in range(L)])
    f2P = np.stack([_sb_pack(ws * g_f2w[i, l].T, f8np) for l in range(L)])

    par = np.zeros((128, 128), np.float32)
    par[:, _P_IPB:_P_IPB + KC] = _pack_pcol(g_ipb[i])
    for l in range(L):
        pb = _P_LAYER + 40 * l
        bal = ba[l] + (g_ipb[i] if l == 0 else 0.0)   # layer-0 fuses ipb
        par[:, pb:pb + 4] = _pack_pcol(bal)
        par[:, pb + 4:pb + 20] = _pack_pcol(g_f1b[i, l])
        par[:, pb + 20:pb + 24] = _pack_pcol(g_f2b[i, l])
        par[:, pb + 24:pb + 28] = _pack_pcol(g_ln1g[i, l])
        par[:, pb + 28:pb + 32] = _pack_pcol(g_ln1b[i, l])
        par[:, pb + 32:pb + 36] = _pack_pcol(g_ln2g[i, l])
        par[:, pb + 36:pb + 40] = _pack_pcol(g_ln2b[i, l])
    par[:, _P_OPB:_P_OPB + KC] = _pack_pcol((1.0 - rw) * g_opb[i])

    zb = bool(np.all(g_ipb[i] == 0) and all(np.all(b == 0) for b in ba)
              and np.all(g_f1b[i] == 0) and np.all(g_f2b[i] == 0)
              and np.all(g_opb[i] == 0))
    ug = bool(np.all(g_ln1g[i] == 1) and np.all(g_ln2g[i] == 1))
    zbeta = bool(np.all(g_ln1b[i] == 0) and np.all(g_ln2b[i] == 0))
    return {"ip": ipP, "op": opP, "wa": waP, "f1": f1P, "f2": f2P,
            "par": par}, (zb, ug, zbeta), rw


def _prepare(inputs):
    """Host-side prep. Returns (nc, in_maps, assemble)."""
    image = np.asarray(inputs["image_features"], np.float32)
    text = np.asarray(inputs["text_features"], np.float32)
    mt = np.asarray(inputs["missing_type"])

    idx1 = np.nonzero(mt == 1)[0]      # gen0 (img -> text) fills text
    idx2 = np.nonzero(mt == 2)[0]      # gen1 (text -> img) fills img
    idx3 = np.nonzero(mt == 3)[0]

    gw = {k: np.asarray(v) for k, v in inputs.items() if k.startswith("g_")}
    w0, skips0, rw0 = _prep_gen_weights(0, **gw)
    w1, skips1, rw1 = _prep_gen_weights(1, **gw)
    skips = tuple(a and b for a, b in zip(skips0, skips1))

    # prior MLP on host (tiny)
    pe = np.asarray(inputs["prior_emb"], np.float64)
    t = pe @ np.asarray(inputs["prior_w1"], np.float64).T \
        + np.asarray(inputs["prior_b1"], np.float64)
    t = 0.5 * t * (1.0 + np.vectorize(math.erf)(t / math.sqrt(2.0)))
    prior = (t @ np.asarray(inputs["prior_w2"], np.float64).T
             + np.asarray(inputs["prior_b2"], np.float64)).astype(np.float32)
    p_img, p_text = prior[0, :H], prior[0, H:]

    imgT = np.ascontiguousarray(image.T)
    textT = np.ascontiguousarray(text.T)

    n_pc = -(-max(len(idx1), len(idx2), 1) // GCORES)   # per-core columns
    C = max(64, -(-n_pc // 64) * 64)                    # round up to 64

    tls = _tiles(C)

    def _pack_tm(M):
        """[H, C] -> tile-major [128, NT*KC*Tt] fp8."""
        a = M.astype(mybir.dt.np(f8)).reshape(KC, 128, C).transpose(1, 0, 2)
        return np.concatenate(
            [np.ascontiguousarray(a[:, :, t0:t1]).reshape(128, -1)
             for t0, t1 in tls], axis=1)

    def shard_cols(Tsrc, Ttgt, idx):
        pad = np.zeros(GCORES * C, np.int64)
        pad[:len(idx)] = idx
        pad = pad.reshape(GCORES, C)
        return [_pack_tm(Tsrc[:, pad[c]]) for c in range(GCORES)], \
            [_pack_tm(Ttgt[:, pad[c]]) for c in range(GCORES)]

    src0, tgt0 = shard_cols(imgT, textT, idx1)
    src1, tgt1 = shard_cols(textT, imgT, idx2)

    nc = _get_program(C, skips)

    ones = np.ones((128, 128), ml_dtypes.bfloat16)
    ident = (np.eye(128, dtype=np.float32) * W8SCALE).astype(ml_dtypes.bfloat16)
    in_maps = []
    for c in range(N_CORES):
        g = 0 if c < GCORES else 1
        w = w0 if g == 0 else w1
        lc = c % GCORES
        in_maps.append({
            "src": (src0 if g == 0 else src1)[lc],
            "tgt": (tgt0 if g == 0 else tgt1)[lc],
            "ip": w["ip"], "op": w["op"], "wa": w["wa"], "f1": w["f1"],
            "f2": w["f2"],
            "par": w["par"], "ones": ones, "ident": ident,
        })

    def assemble(results):
        def gather_out(cores, idx, rw, full):
            cols = [np.asarray(results[c]["out"])
                    .astype(np.float32)
                    .reshape(128, KC, C).transpose(1, 0, 2).reshape(H, C)
                    for c in cores]
            allc = np.concatenate(cols, axis=1)[:, :len(idx)]
            return rw * full[idx] + allc.T

        enhanced_text = text.copy()
        if len(idx1):
            enhanced_text[idx1] = gather_out(range(GCORES), idx1, rw0, text)
        enhanced_img = image.copy()
        if len(idx2):
            enhanced_img[idx2] = gather_out(range(GCORES, N_CORES), idx2,
                                            rw1, image)
        if len(idx3):
            enhanced_img[idx3] = p_img
            enhanced_text[idx3] = p_text
        return enhanced_img, enhanced_text

    return nc, in_maps, assemble


def kernel(**inputs):
    nc, in_maps, assemble = _prepare(inputs)
    res = run_bass_kernel_spmd(nc, in_maps, list(range(N_CORES)))
    return assemble(res.results)
